# revision 1
# baseline (speedup 1.0000x reference)
"""GCATopo (2-layer GTAT GNN) Trainium2 kernel, 8-way SPMD.

Strategy:
 - Nodes partitioned into 8 contiguous ranges (one per core). Edges are
   assigned to the core that owns their dst node, sorted by dst, padded so
   every 128-dst-node block starts at a fresh 128-edge tile. Per-block tile
   counts are shared across cores (SPMD: one program, per-core data).
 - Per layer, each core computes for its node slice a packed "ext" row
   [xl(512) | topo(15) | 1.0 | al(4) | ta(4) | pad] = 576 f32 (2304B) via
   dense matmuls (attention logit weights folded into the same matmuls), then
   an AllGather replicates the ext table to every core (the halo exchange).
 - Edge phase: dma_gather pulls src-node ext rows per 128-edge tile; edge
   attention weights are computed in-register; the segment-sum over dst runs
   on the tensor engine via one-hot selection matrices (S[e,d] = (dst==d))
   accumulating into PSUM per dst block; softmax normalization (and its
   denominator, accumulated alongside) is applied after aggregation.
 - Final: mean-pool partials + AllReduce + tiny MLP heads on every core.
"""

from contextlib import ExitStack

import numpy as np

import concourse.bass as bass
import concourse.bacc as bacc
import concourse.tile as tile
from concourse import mybir
from concourse.masks import make_identity
from concourse.bass_utils import run_bass_kernel_spmd

F32 = mybir.dt.float32
BF16 = mybir.dt.bfloat16
I16 = mybir.dt.int16
AF = mybir.ActivationFunctionType
OP = mybir.AluOpType

P = 128


class Cfg:
    def __init__(self, N=20000, E=240000, FIN=576, HID=128, TOPO=15, H=4,
                 CORES=8, NEG=0.2):
        self.N, self.E, self.FIN, self.HID, self.TOPO, self.H = N, E, FIN, HID, TOPO, H
        self.CORES, self.NEG = CORES, NEG
        self.HC = H * HID                      # 512
        self.ROW = self.HC + 128               # packed ext row (bf16), 1280B
        assert (self.ROW * 2) % 256 == 0
        self.NPC = N // CORES                  # nodes per core
        self.NBLK = (self.NPC + P - 1) // P    # dst blocks per core
        # f32 payload lives in bf16 slots [HC : HC+64) -> f32 view [0:32)
        self.C_TOPO = 0                        # in the f32 view
        self.C_ONE = TOPO
        self.C_AL = 16
        self.C_TA = 20
        self.DATT = 64                         # dst_att row (f32), 256B


CFG = Cfg()
GT_MAX = 8  # max tiles (=128 idxs each) per gather call; HW rejects >~1k idxs


def cdiv(a, b):
    return (a + b - 1) // b


def ktiles(F):
    return [(o, min(P, F - o)) for o in range(0, F, P)]


# --------------------------------------------------------------------------
# host-side graph preprocessing (pure indexing on edge_index)
# --------------------------------------------------------------------------

def host_prep(edge_index, cfg):
    N, CORES, NPC, NBLK = cfg.N, cfg.CORES, cfg.NPC, cfg.NBLK
    src = np.asarray(edge_index[0], dtype=np.int64)
    dst = np.asarray(edge_index[1], dtype=np.int64)
    loops = np.arange(N, dtype=np.int64)
    src = np.concatenate([src, loops])
    dst = np.concatenate([dst, loops])
    order = np.argsort(dst, kind="stable")
    s, d = src[order], dst[order]

    core_of = d // NPC
    blk_of = (d % NPC) // P
    counts = np.zeros((CORES, NBLK), dtype=np.int64)
    for c in range(CORES):
        m = core_of == c
        bb = blk_of[m]
        for b in range(NBLK):
            counts[c, b] = int((bb == b).sum())
    schedule = [max(1, cdiv(int(counts[:, b].max()), P)) for b in range(NBLK)]
    offs = np.concatenate([[0], np.cumsum(schedule)]).astype(np.int64)
    ttot = int(offs[-1])

    srcidx = np.zeros((CORES, ttot * P), dtype=np.int16)
    dstidx = np.zeros((CORES, ttot * P), dtype=np.int16)
    dstloc = np.full((CORES, ttot * P), -1.0, dtype=np.float32)
    for c in range(CORES):
        m = core_of == c
        sc, dc, bc = s[m], d[m], blk_of[m]
        for b in range(NBLK):
            mb = bc == b
            n = int(mb.sum())
            base = int(offs[b]) * P
            srcidx[c, base:base + n] = sc[mb].astype(np.int16)
            dstidx[c, base:base + n] = (dc[mb] - c * NPC).astype(np.int16)
            dstloc[c, base:base + n] = (dc[mb] - (c * NPC + b * P)).astype(np.float32)

    # wrap for dma_gather: index i lives at [i % 16, i // 16]; the 16-row
    # block is replicated 8x along partitions (one stripe per gpsimd core)
    src_w = [np.tile(srcidx[c].reshape(-1, 16).T, (8, 1)).copy()
             for c in range(CORES)]
    dst_w = [np.tile(dstidx[c].reshape(-1, 16).T, (8, 1)).copy()
             for c in range(CORES)]
    # dstloc: edge j -> tile j//128, partition j%128
    dl_m = [dstloc[c].reshape(ttot, P).T.copy() for c in range(CORES)]
    return schedule, src_w, dst_w, dl_m


def host_attB(att, cfg):
    """att [1,H,C] -> block-diag [H*C, H] (pure placement of input values)."""
    H, C = cfg.H, cfg.HID
    out = np.zeros((H * C, H), dtype=np.float32)
    a = np.asarray(att, dtype=np.float32).reshape(H, C)
    for h in range(H):
        out[h * C:(h + 1) * C, h] = a[h]
    return out


# --------------------------------------------------------------------------
# program builder
# --------------------------------------------------------------------------

class Prog:
    pass


def build_program(cfg, schedule):
    es = ExitStack()
    nc = bacc.Bacc("TRN2", target_bir_lowering=False, debug=False,
                   num_devices=cfg.CORES)
    pr = Prog()
    pr.nc = nc
    N, FIN, HID, TOPO, H, HC, ROW, NPC, NBLK = (
        cfg.N, cfg.FIN, cfg.HID, cfg.TOPO, cfg.H, cfg.HC, cfg.ROW, cfg.NPC,
        cfg.NBLK)
    TTOT = sum(schedule)
    TMAX = max(schedule)
    W16 = TTOT * P // 16
    groups = [list(range(cfg.CORES))]
    SMW = 4 * 16 + H  # 68

    def din(name, shape, dtype=F32):
        return nc.dram_tensor(name, list(shape), dtype, kind="ExternalInput")

    # ---- external inputs ----
    x_sl = din("x_slice", (NPC, FIN))
    te_w1 = din("te_w1", (FIN, HID)); te_b1 = din("te_b1", (HID,))
    te_w2 = din("te_w2", (HID, TOPO)); te_b2 = din("te_b2", (TOPO,))
    wts = {}
    for L in (1, 2):
        KIN = FIN if L == 1 else HC
        wts[L] = dict(
            wl=din(f"l{L}_wl", (KIN, HC)), bl=din(f"l{L}_bl", (HC,)),
            wr=din(f"l{L}_wr", (KIN, HC)), br=din(f"l{L}_br", (HC,)),
            attB=din(f"l{L}_attB", (HC, H)), att2T=din(f"l{L}_att2T", (TOPO, H)),
            bias=din(f"l{L}_bias", (HC,)), bias2=din(f"l{L}_bias2", (TOPO,)),
        )
    heads = {}
    for nm in ("v", "a"):
        heads[nm] = dict(w1=din(f"{nm}_w1", (HC, HID)), b1=din(f"{nm}_b1", (HID,)),
                         w2=din(f"{nm}_w2", (HID, 1)), b2=din(f"{nm}_b2", (1,)))
    src_i = din("src_idx", (P, W16), I16)
    dst_i = din("dst_idx", (P, W16), I16)
    dl_i = din("dstloc", (P, TTOT))

    # ---- outputs ----
    val_o = nc.dram_tensor("valence", [1, 1], F32, kind="ExternalOutput")
    aro_o = nc.dram_tensor("arousal", [1, 1], F32, kind="ExternalOutput")

    # ---- internal DRAM ----
    ext_sl = [nc.dram_tensor(f"ext_slice{L}", [NPC, ROW], BF16)
              for L in (1, 2)]
    ext_fl = [nc.dram_tensor(f"ext_full{L}", [N, ROW], BF16,
                             addr_space="Shared") for L in (1, 2)]
    datt_sl = [nc.dram_tensor(f"datt_slice{L}", [NPC, cfg.DATT], F32)
               for L in (1, 2)]
    pool_in = nc.dram_tensor("pool_in", [P, H], F32)
    pool_out = nc.dram_tensor("pool_out", [P, H], F32, addr_space="Shared")

    blocks = ktiles(NPC)          # node chunks (== dst blocks) per core
    fkt = ktiles(FIN)
    ckt = ktiles(HC)
    offs = np.concatenate([[0], np.cumsum(schedule)]).astype(int)

    ctx_noncontig = nc.allow_non_contiguous_dma("tiny transposed weight loads")
    ctx_noncontig.__enter__()
    with tile.TileContext(nc) as tc:
        # ================= static SBUF =================
        ident = nc.alloc_sbuf_tensor("ident", [P, P], F32).ap()
        make_identity(nc, ident)
        iota_i = nc.alloc_sbuf_tensor("iota_i", [P, P], mybir.dt.int32).ap()
        nc.gpsimd.iota(iota_i, pattern=[[1, P]], base=0, channel_multiplier=0)
        iota_row = nc.alloc_sbuf_tensor("iota_row", [P, P], F32).ap()
        nc.vector.tensor_copy(iota_row, iota_i)
        iota_bf = nc.alloc_sbuf_tensor("iota_bf", [P, P], BF16).ap()
        nc.vector.tensor_copy(iota_bf, iota_i)
        ones128 = nc.alloc_sbuf_tensor("ones128", [P, P], F32).ap()
        nc.gpsimd.memset(ones128, 1.0)
        ones_row = nc.alloc_sbuf_tensor("ones_row", [1, NPC], F32).ap()
        nc.gpsimd.memset(ones_row, 1.0)

        src_sb = nc.alloc_sbuf_tensor("src_sb", [P, W16], I16).ap()
        dst_sb = nc.alloc_sbuf_tensor("dst_sb", [P, W16], I16).ap()
        dl_sb = nc.alloc_sbuf_tensor("dl_sb", [P, TTOT], F32).ap()
        nc.sync.dma_start(src_sb, src_i[:, :])
        nc.sync.dma_start(dst_sb, dst_i[:, :])
        nc.sync.dma_start(dl_sb, dl_i[:, :])
        dl_bf = nc.alloc_sbuf_tensor("dl_bf", [P, TTOT], BF16).ap()
        nc.vector.tensor_copy(dl_bf, dl_sb)

        topoT = [nc.alloc_sbuf_tensor(f"topoT{L}", [TOPO, NPC], F32).ap()
                 for L in (1, 2)]
        hfm = [nc.alloc_sbuf_tensor(f"hfm{h}", [P, NPC], F32).ap()
               for h in range(H)]
        pool_parts = nc.alloc_sbuf_tensor("pool_parts", [P, H * NBLK], F32).ap()
        b2col = nc.alloc_sbuf_tensor("b2col", [TOPO, 1], F32).ap()
        nc.sync.dma_start(b2col, wts[1]["bias2"][:].rearrange("(t o) -> t o", o=1))
        bcol = {}
        for L in (1, 2):
            bcol[L] = nc.alloc_sbuf_tensor(f"bcol{L}", [P, H], F32).ap()
            nc.sync.dma_start(bcol[L],
                              wts[L]["bias"][:].rearrange("(h c) -> c h", h=H))

        # =========================================================
        def emit_aux_weights(w, KIN, wl_sb, wp, pp):
            """[wl@attB | wr@attB] k-tiles [(fk,8)] + bias row [1,8].

            wl_sb: already-loaded main wl k-tiles (reused as transpose src).
            Layout: cols 0:4 = al weights, 4:8 = ar weights.
            """
            kk = ktiles(KIN)
            nk = len(kk)
            with tc.tile_pool(name=f"aux{KIN}", bufs=2) as ap_:
                pw = pp.tile([P, nk * 8 + 8], F32, tag="auxw", name="auxw", space="PSUM", bufs=1)
                blc = wp.tile([P, HC // P], F32, tag="blc", name="blc")
                nc.sync.dma_start(blc[:, :],
                                  w["bl"][:].rearrange("(a c) -> c a", c=P))
                brc = wp.tile([P, HC // P], F32, tag="brc", name="brc")
                nc.sync.dma_start(brc[:, :],
                                  w["br"][:].rearrange("(a c) -> c a", c=P))
                for ci, (co, ck) in enumerate(ckt):
                    sp_ = ci == len(ckt) - 1
                    attB_t = ap_.tile([P, H], F32, tag="attB", name="attB")
                    nc.sync.dma_start(attB_t[:ck, :], w["attB"][co:co + ck, :])
                    for wsel in (0, 1):  # 0 = wl, 1 = wr
                        wT = ap_.tile([P, KIN], F32, tag=f"wT{wsel}", name=f"wT{wsel}")
                        for fi, (fo, fk) in enumerate(kk):
                            if wsel == 0:
                                src_ap = wl_sb[fi][:fk, co:co + ck]
                            else:
                                tmp = ap_.tile([P, P], F32, tag="wrs", name="wrs")
                                nc.sync.dma_start(tmp[:fk, :ck],
                                                  w["wr"][fo:fo + fk, co:co + ck])
                                src_ap = tmp[:fk, :ck]
                            pt = pp.tile([P, P], F32, tag="tt", name="tt", space="PSUM")
                            nc.tensor.transpose(pt[:ck, :fk], src_ap,
                                                ident[:fk, :fk])
                            nc.vector.tensor_copy(wT[:ck, fo:fo + fk],
                                                  pt[:ck, :fk])
                        for fi, (fo, fk) in enumerate(kk):
                            cs = fi * 8 + 4 * wsel
                            st = ci == 0 and fi == 0 and wsel == 0
                            nc.tensor.matmul(pw[:fk, cs:cs + 4],
                                             lhsT=wT[:ck, fo:fo + fk],
                                             rhs=attB_t[:ck, :], start=st,
                                             stop=sp_, skip_group_check=True)
                        bc_ = blc if wsel == 0 else brc
                        nc.tensor.matmul(pw[:1, nk * 8 + 4 * wsel:
                                                nk * 8 + 4 * wsel + 4],
                                         lhsT=bc_[:ck, ci:ci + 1],
                                         rhs=attB_t[:ck, :], start=False,
                                         stop=sp_, skip_group_check=True)
                alar = []
                for fi, (fo, fk) in enumerate(kk):
                    t = wp.tile([P, 8], F32, tag=f"alar{fo}", name=f"alar{fo}")
                    nc.vector.tensor_copy(t[:fk, :], pw[:fk, fi * 8:fi * 8 + 8])
                    alar.append(t)
                alar_b = wp.tile([1, 8], F32, tag="alar_b", name="alar_b")
                nc.vector.tensor_copy(alar_b[:, :], pw[:1, nk * 8:nk * 8 + 8])
            return alar, alar_b

        # =========================================================
        def emit_prep(L, featT, wp, pp, cp):
            """featT: list of (ap, k) feat-major k-tiles [k, NPC]. Emits
            ext_slice + datt_slice for layer L, then the AllGather."""
            w = wts[L]
            KIN = FIN if L == 1 else HC
            kk = ktiles(KIN)
            wl_sb = []
            for (fo, fk) in kk:
                t = wp.tile([P, HC], F32, tag=f"wl{fo}", name=f"wl{fo}")
                nc.sync.dma_start(t[:fk, :], w["wl"][fo:fo + fk, :])
                wl_sb.append(t)
            blrow = wp.tile([1, HC], F32, tag="blrow", name="blrow")
            nc.sync.dma_start(blrow[:, :], w["bl"][:].rearrange("(o c) -> o c", o=1))
            alar, alar_b = emit_aux_weights(w, KIN, wl_sb, wp, pp)
            att2T_sb = wp.tile([TOPO, H], F32, tag="att2T", name="att2T")
            nc.sync.dma_start(att2T_sb[:, :], w["att2T"][:, :])

            tT = topoT[L - 1]
            for bi, (bo, bs) in enumerate(blocks):
                pm = pp.tile([P, HC], F32, tag="main", name="main", space="PSUM")
                pa = pp.tile([P, 2 * H], F32, tag="aux", name="aux", space="PSUM")
                pta = pp.tile([P, H], F32, tag="ta", name="ta", space="PSUM", bufs=1)
                for i in range(len(kk)):
                    ft, k = featT[i]
                    nc.tensor.matmul(pm[:bs, :], lhsT=ft[:k, bo:bo + bs],
                                     rhs=wl_sb[i][:k, :], start=i == 0,
                                     stop=False, skip_group_check=True)
                    nc.tensor.matmul(pa[:bs, :], lhsT=ft[:k, bo:bo + bs],
                                     rhs=alar[i][:k, :], start=i == 0,
                                     stop=False, skip_group_check=True)
                nc.tensor.matmul(pm[:bs, :], lhsT=ones_row[:, bo:bo + bs],
                                 rhs=blrow[:, :], start=False, stop=True,
                                 skip_group_check=True)
                nc.tensor.matmul(pa[:bs, :], lhsT=ones_row[:, bo:bo + bs],
                                 rhs=alar_b[:, :], start=False, stop=True,
                                 skip_group_check=True)
                nc.tensor.matmul(pta[:bs, :], lhsT=tT[:, bo:bo + bs],
                                 rhs=att2T_sb[:, :], start=True, stop=True)
                ext = cp.tile([P, ROW], BF16, tag="ext", name="ext")
                nc.vector.memset(ext[:, HC + 64:ROW], 0.0)
                nc.vector.tensor_copy(ext[:bs, 0:HC], pm[:bs, :])
                extf = ext[:, HC:HC + 64].bitcast(F32)
                nc.vector.memset(extf[:, 24:32], 0.0)
                ptt = pp.tile([P, P], F32, tag="tt", name="tt", space="PSUM")
                nc.tensor.transpose(ptt[:bs, :TOPO], tT[:, bo:bo + bs],
                                    ident[:TOPO, :TOPO])
                nc.scalar.copy(extf[:bs, cfg.C_TOPO:cfg.C_TOPO + TOPO],
                               ptt[:bs, :TOPO])
                nc.vector.memset(extf[:bs, cfg.C_ONE:cfg.C_ONE + 1], 1.0)
                nc.scalar.copy(extf[:bs, cfg.C_AL:cfg.C_AL + H], pa[:bs, 0:H])
                nc.scalar.copy(extf[:bs, cfg.C_TA:cfg.C_TA + H], pta[:bs, :])
                nc.sync.dma_start(ext_sl[L - 1][bo:bo + bs, :], ext[:bs, :])
                datt = cp.tile([P, cfg.DATT], F32, tag="datt", name="datt")
                nc.vector.memset(datt[:, 2 * H:cfg.DATT], 0.0)
                nc.scalar.copy(datt[:bs, 0:H], pa[:bs, H:2 * H])
                nc.scalar.copy(datt[:bs, H:2 * H], pta[:bs, :])
                nc.sync.dma_start(datt_sl[L - 1][bo:bo + bs, :],
                                  datt[:bs, :])
            nc.gpsimd.collective_compute(
                "AllGather", OP.bypass, replica_groups=groups,
                ins=[ext_sl[L - 1][:, :]], outs=[ext_fl[L - 1][:, :]])

        # =========================================================
        def emit_edge(L, gp, sp, pp):
            """Edge phase for layer L: fills hfm+topoT[1] (L=1) or
            pool_parts (L=2)."""
            for bi, (bo, bs) in enumerate(blocks):
                Tb = schedule[bi]
                base = int(offs[bi])
                gtiles = []
                for go in range(0, Tb, GT_MAX):
                    gn = min(GT_MAX, Tb - go)
                    g = gp.tile([P, GT_MAX * ROW], BF16, tag="G", name="G")
                    c0 = (base + go) * 8
                    nc.gpsimd.dma_gather(
                        g[:, 0:gn * ROW].rearrange("p (t e) -> p t e", e=ROW),
                        ext_fl[L - 1][:, :], src_sb[:, c0:c0 + 8 * gn],
                        num_idxs=P * gn, num_idxs_reg=P * gn, elem_size=ROW,
                        queue_num=0)
                    gtiles.append((go, gn, g))
                dt = gp.tile([P, TMAX * cfg.DATT], F32, tag="D", name="D")
                for go in range(0, Tb, GT_MAX):
                    gn = min(GT_MAX, Tb - go)
                    c0 = (base + go) * 8
                    nc.gpsimd.dma_gather(
                        dt[:, go * cfg.DATT:(go + gn) * cfg.DATT].rearrange(
                            "p (t e) -> p t e", e=cfg.DATT),
                        datt_sl[L - 1][:, :], dst_sb[:, c0:c0 + 8 * gn],
                        num_idxs=P * gn, num_idxs_reg=P * gn,
                        elem_size=cfg.DATT, queue_num=0)
                pfh = [pp.tile([P, P], F32, tag=f"feat{h}", name=f"feat{h}",
                               space="PSUM", bufs=1) for h in range(H)]
                psm = pp.tile([SMW, P], F32, tag="small", name="small",
                              space="PSUM", bufs=1)
                for t in range(Tb):
                    go, gn, g = gtiles[t // GT_MAX]
                    lt = t - go
                    G = g[:, lt * ROW:(lt + 1) * ROW]
                    D = dt[:, t * cfg.DATT:t * cfg.DATT + 2 * H]
                    Gf = G[:, HC:HC + 64].bitcast(F32)
                    lg = sp.tile([P, 2 * H], F32, tag="lg", name="lg")
                    nc.vector.tensor_tensor(
                        lg, Gf[:, cfg.C_AL:cfg.C_AL + 2 * H], D, OP.add)
                    lr = sp.tile([P, 2 * H], F32, tag="lr", name="lr")
                    nc.vector.tensor_scalar(lr, lg, cfg.NEG, None, OP.mult)
                    nc.vector.tensor_tensor(lr, lr, lg, OP.max)
                    et = sp.tile([P, 2 * H], F32, tag="et", name="et")
                    nc.scalar.activation(et, lr, AF.Exp)
                    ebf = sp.tile([P, 2 * H], BF16, tag="ebf", name="ebf")
                    nc.scalar.copy(ebf, et)
                    St = sp.tile([P, P], BF16, tag="S", name="S")
                    nc.vector.tensor_tensor(
                        St, iota_bf,
                        dl_bf[:, base + t:base + t + 1].to_broadcast((P, P)),
                        OP.is_equal)
                    st0, sp1 = t == 0, t == Tb - 1
                    if L == 1:
                        SM = sp.tile([P, SMW], BF16, tag="SM", name="SM")
                        for h in range(H):
                            nc.vector.tensor_scalar(
                                SM[:, 16 * h:16 * h + 16],
                                Gf[:, cfg.C_TOPO:cfg.C_TOPO + 16],
                                et[:, h:h + 1], None, OP.mult)
                        nc.scalar.copy(SM[:, 64:64 + H], et[:, H:2 * H])
                        nc.tensor.matmul(psm[:, :], lhsT=SM[:, :], rhs=St,
                                         start=st0, stop=sp1,
                                         skip_group_check=True)
                    else:
                        nc.tensor.matmul(psm[:H, :], lhsT=ebf[:, H:2 * H],
                                         rhs=St, start=st0, stop=sp1,
                                         skip_group_check=True)
                    Gp = sp.tile([P, HC], BF16, tag="Gp", name="Gp")
                    for h in range(H):
                        sl = slice(h * P, (h + 1) * P)
                        if h % 2 == 0:
                            nc.vector.tensor_scalar(Gp[:, sl], G[:, sl],
                                                    et[:, H + h:H + h + 1],
                                                    None, OP.mult)
                        else:
                            nc.scalar.activation(Gp[:, sl], G[:, sl], AF.Copy,
                                                 scale=et[:, H + h:H + h + 1])
                        nc.tensor.matmul(pfh[h][:, :], lhsT=Gp[:, sl],
                                         rhs=St, start=st0, stop=sp1,
                                         skip_group_check=True)
                # ---- drain block ----
                sm_sb = sp.tile([SMW, P], F32, tag="sm_sb", name="sm_sb")
                ts_sb = sp.tile([P, SMW], F32, tag="ts_sb", name="ts_sb")
                recS = sp.tile([P, SMW], F32, tag="recS", name="recS")
                ptt = pp.tile([P, P], F32, tag="tt", name="tt", space="PSUM")
                # clamp away the exact zeros of unused dst slots in partial
                # blocks before reciprocal (0 -> inf -> 0*inf = NaN poison)
                tsafe = sp.tile([P, SMW], F32, tag="tsafe", name="tsafe")
                if L == 1:
                    nc.vector.tensor_copy(sm_sb, psm[:, :])
                    nc.tensor.transpose(ptt[:, :SMW], sm_sb,
                                        ident[:SMW, :SMW])
                    nc.vector.tensor_copy(ts_sb, ptt[:, :SMW])
                    nc.vector.tensor_scalar(tsafe, ts_sb, 1e-30, None, OP.max)
                    nc.vector.reciprocal(recS, tsafe)
                    rec2 = recS[:, 64:64 + H]
                else:
                    nc.vector.tensor_copy(sm_sb[:H, :], psm[:H, :])
                    nc.tensor.transpose(ptt[:, :H], sm_sb[:H, :],
                                        ident[:H, :H])
                    nc.vector.tensor_copy(ts_sb[:, :H], ptt[:, :H])
                    nc.vector.tensor_scalar(tsafe[:, :H], ts_sb[:, :H], 1e-30,
                                            None, OP.max)
                    nc.vector.reciprocal(recS[:, :H], tsafe[:, :H])
                    rec2 = recS[:, 0:H]
                pdv = pp.tile([P, H * P], F32, tag="div", name="div",
                              space="PSUM", bufs=1)
                dsb = sp.tile([P, H * P], F32, tag="dsb", name="dsb")
                for h in range(H):
                    dg = sp.tile([P, P], F32, tag="diag", name="diag")
                    nc.vector.tensor_scalar(dg, ident, rec2[:, h:h + 1], None,
                                            OP.mult)
                    nc.tensor.matmul(pdv[:, h * P:(h + 1) * P],
                                     lhsT=ones128, rhs=dg, start=True,
                                     stop=True, skip_group_check=True)
                nc.vector.tensor_copy(dsb, pdv[:, :])
                if L == 1:
                    for h in range(H):
                        sl = slice(h * P, h * P + bs)
                        nc.vector.tensor_tensor(hfm[h][:, bo:bo + bs],
                                                pfh[h][:, :bs], dsb[:, sl],
                                                OP.mult)
                        nc.vector.tensor_scalar(hfm[h][:, bo:bo + bs],
                                                hfm[h][:, bo:bo + bs],
                                                bcol[1][:, h:h + 1], None,
                                                OP.add)
                    tp = sp.tile([P, TOPO], F32, tag="tp", name="tp")
                    nc.vector.tensor_scalar(tp, ts_sb[:, 0:TOPO],
                                            recS[:, 15:16], None, OP.mult)
                    for h in range(1, H):
                        tp2 = sp.tile([P, TOPO], F32, tag="tp2", name="tp2")
                        nc.vector.tensor_scalar(tp2,
                                                ts_sb[:, 16 * h:16 * h + TOPO],
                                                recS[:, 16 * h + 15:16 * h + 16],
                                                None, OP.mult)
                        nc.vector.tensor_tensor(tp, tp, tp2, OP.add)
                    pt2 = pp.tile([P, P], F32, tag="tt", name="tt", space="PSUM")
                    nc.tensor.transpose(pt2[:TOPO, :], tp, ident)
                    nc.vector.tensor_scalar(topoT[1][:, bo:bo + bs],
                                            pt2[:TOPO, :bs], 1.0 / H,
                                            b2col[:, 0:1], OP.mult, OP.add)
                else:
                    for h in range(H):
                        sl = slice(h * P, h * P + bs)
                        pm2 = sp.tile([P, P], F32, tag="pm", name="pm")
                        nc.vector.tensor_tensor(pm2[:, :bs], pfh[h][:, :bs],
                                                dsb[:, sl], OP.mult)
                        nc.vector.tensor_reduce(
                            pool_parts[:, h * NBLK + bi:h * NBLK + bi + 1],
                            pm2[:, :bs], mybir.AxisListType.X, OP.add)

        # ================= phase A: layer-1 prep =================
        with tc.tile_pool(name="wpA", bufs=1) as wpA, \
             tc.tile_pool(name="ppA", bufs=2, space="PSUM") as ppA, \
             tc.tile_pool(name="cpA", bufs=3) as cpA, \
             tc.tile_pool(name="xpA", bufs=1) as xpA:
            xT = [xpA.tile([P, NPC], F32, tag=f"xT{fo}", name=f"xT{fo}") for (fo, fk) in fkt]
            for bi, (bo, bs) in enumerate(blocks):
                xc = cpA.tile([P, FIN], F32, tag="xc", name="xc")
                nc.sync.dma_start(xc[:bs, :], x_sl[bo:bo + bs, :])
                for fi, (fo, fk) in enumerate(fkt):
                    pt = ppA.tile([P, P], F32, tag="tt", name="tt", space="PSUM")
                    nc.tensor.transpose(pt[:fk, :bs], xc[:bs, fo:fo + fk],
                                        ident[:bs, :bs])
                    nc.vector.tensor_copy(xT[fi][:fk, bo:bo + bs],
                                          pt[:fk, :bs])
            tw1 = []
            for (fo, fk) in fkt:
                t = wpA.tile([P, HID], F32, tag=f"tw1{fo}", name=f"tw1{fo}")
                nc.sync.dma_start(t[:fk, :], te_w1[fo:fo + fk, :])
                tw1.append(t)
            tb1r = wpA.tile([1, HID], F32, tag="tb1r", name="tb1r")
            nc.sync.dma_start(tb1r[:, :], te_b1[:].rearrange("(o c) -> o c", o=1))
            tw2 = wpA.tile([HID, TOPO], F32, tag="tw2", name="tw2")
            nc.sync.dma_start(tw2[:, :], te_w2[:, :])
            tb2r = wpA.tile([1, TOPO], F32, tag="tb2r", name="tb2r")
            nc.sync.dma_start(tb2r[:, :], te_b2[:].rearrange("(o c) -> o c", o=1))
            t_hid = xpA.tile([P, NPC], F32, tag="t_hid", name="t_hid")
            NG = 512
            for go in range(0, NPC, NG):
                gs = min(NG, NPC - go)
                ph = ppA.tile([P, NG], F32, tag="main", name="main", space="PSUM")
                for fi, (fo, fk) in enumerate(fkt):
                    nc.tensor.matmul(ph[:, :gs], lhsT=tw1[fi][:fk, :],
                                     rhs=xT[fi][:fk, go:go + gs],
                                     start=fi == 0, stop=False,
                                     skip_group_check=True)
                nc.tensor.matmul(ph[:, :gs], lhsT=tb1r[:, :],
                                 rhs=ones_row[:, go:go + gs], start=False,
                                 stop=True, skip_group_check=True)
                nc.scalar.activation(t_hid[:, go:go + gs], ph[:, :gs], AF.Relu)
                pt = ppA.tile([P, NG], F32, tag="main", name="main", space="PSUM")
                nc.tensor.matmul(pt[:TOPO, :gs], lhsT=tw2[:, :],
                                 rhs=t_hid[:, go:go + gs], start=True,
                                 stop=False, skip_group_check=True)
                nc.tensor.matmul(pt[:TOPO, :gs], lhsT=tb2r[:, :],
                                 rhs=ones_row[:, go:go + gs], start=False,
                                 stop=True, skip_group_check=True)
                nc.vector.tensor_copy(topoT[0][:, go:go + gs], pt[:TOPO, :gs])
            featT1 = [(xT[i], fkt[i][1]) for i in range(len(fkt))]
            emit_prep(1, featT1, wpA, ppA, cpA)

        # ================= phase B: layer-1 edges =================
        with tc.tile_pool(name="gpB", bufs=2) as gpB, \
             tc.tile_pool(name="spB", bufs=4) as spB, \
             tc.tile_pool(name="ppB", bufs=2, space="PSUM") as ppB:
            emit_edge(1, gpB, spB, ppB)

        # ================= phase C: layer-2 prep =================
        with tc.tile_pool(name="wpC", bufs=1) as wpC, \
             tc.tile_pool(name="ppC", bufs=2, space="PSUM") as ppC, \
             tc.tile_pool(name="cpC", bufs=3) as cpC:
            featT2 = [(hfm[h], P) for h in range(H)]
            emit_prep(2, featT2, wpC, ppC, cpC)

        # ================= phase D: layer-2 edges =================
        with tc.tile_pool(name="gpD", bufs=2) as gpD, \
             tc.tile_pool(name="spD", bufs=4) as spD, \
             tc.tile_pool(name="ppD", bufs=2, space="PSUM") as ppD:
            emit_edge(2, gpD, spD, ppD)

        # ================= phase E: pool + MLP heads =================
        with tc.tile_pool(name="wpE", bufs=1) as wpE, \
             tc.tile_pool(name="ppE", bufs=2, space="PSUM") as ppE:
            pooled = wpE.tile([P, H], F32, tag="pooled", name="pooled")
            for h in range(H):
                nc.vector.tensor_reduce(pooled[:, h:h + 1],
                                        pool_parts[:, h * NBLK:(h + 1) * NBLK],
                                        mybir.AxisListType.X, OP.add)
            nc.sync.dma_start(pool_in[:, :], pooled[:, :])
            nc.gpsimd.collective_compute(
                "AllReduce", OP.add, replica_groups=groups,
                ins=[pool_in[:, :]], outs=[pool_out[:, :]])
            pooled2 = wpE.tile([P, H], F32, tag="pooled2", name="pooled2")
            nc.sync.dma_start(pooled2[:, :], pool_out[:, :])
            pmean = wpE.tile([P, H], F32, tag="pmean", name="pmean")
            for h in range(H):
                nc.vector.tensor_scalar(pmean[:, h:h + 1], pooled2[:, h:h + 1],
                                        1.0 / N, bcol[2][:, h:h + 1], OP.mult,
                                        OP.add)
            for nm, out_t in (("v", val_o), ("a", aro_o)):
                hd = heads[nm]
                w1_sb = []
                for ki in range(H):
                    t = wpE.tile([P, HID], F32, tag=f"{nm}w1{ki}", name=f"{nm}w1{ki}")
                    nc.sync.dma_start(t[:, :], hd["w1"][ki * P:(ki + 1) * P, :])
                    w1_sb.append(t)
                b1r = wpE.tile([1, HID], F32, tag=f"{nm}b1r", name=f"{nm}b1r")
                nc.sync.dma_start(b1r[:, :], hd["b1"][:].rearrange("(o c) -> o c", o=1))
                w2c = wpE.tile([HID, 1], F32, tag=f"{nm}w2c", name=f"{nm}w2c")
                nc.sync.dma_start(w2c[:, :], hd["w2"][:, :])
                b2c = wpE.tile([1, 1], F32, tag=f"{nm}b2c", name=f"{nm}b2c")
                nc.sync.dma_start(b2c[:, :], hd["b2"][:].rearrange("(o c) -> o c", o=1))
                pm = ppE.tile([P, 1], F32, tag="mlp", name="mlp", space="PSUM")
                for ki in range(H):
                    nc.tensor.matmul(pm[:, :], lhsT=w1_sb[ki],
                                     rhs=pmean[:, ki:ki + 1], start=ki == 0,
                                     stop=False, skip_group_check=True)
                nc.tensor.matmul(pm[:, :], lhsT=b1r[:, :],
                                 rhs=ones_row[:, 0:1], start=False, stop=True,
                                 skip_group_check=True)
                hv = wpE.tile([P, 1], F32, tag=f"{nm}hv", name=f"{nm}hv")
                nc.scalar.activation(hv[:, :], pm[:, :], AF.Relu)
                po = ppE.tile([1, 1], F32, tag="mlpo", name="mlpo", space="PSUM")
                nc.tensor.matmul(po[:, :], lhsT=hv[:, :], rhs=w2c[:, :],
                                 start=True, stop=False, skip_group_check=True)
                nc.tensor.matmul(po[:, :], lhsT=b2c[:, :],
                                 rhs=ones_row[:, 0:1], start=False, stop=True,
                                 skip_group_check=True)
                ov = wpE.tile([1, 1], F32, tag=f"{nm}ov", name=f"{nm}ov")
                nc.vector.tensor_copy(ov[:, :], po[:, :])
                nc.sync.dma_start(out_t[:, :], ov[:, :])

    ctx_noncontig.__exit__(None, None, None)
    nc.compile()
    es.close()
    return pr


# --------------------------------------------------------------------------
# entry point
# --------------------------------------------------------------------------

_CACHE = {}


def make_in_maps(inputs, cfg, src_w, dst_w, dl_m):
    x = np.ascontiguousarray(np.asarray(inputs["x"], dtype=np.float32))
    shared = {}
    for k in ("te_w1", "te_b1", "te_w2", "te_b2"):
        shared[k] = np.ascontiguousarray(np.asarray(inputs[k], np.float32))
    for L in (1, 2):
        for k in ("wl", "bl", "wr", "br", "bias", "bias2"):
            shared[f"l{L}_{k}"] = np.ascontiguousarray(
                np.asarray(inputs[f"l{L}_{k}"], np.float32))
        shared[f"l{L}_attB"] = host_attB(inputs[f"l{L}_att"], cfg)
        shared[f"l{L}_att2T"] = np.ascontiguousarray(
            np.asarray(inputs[f"l{L}_att2"], np.float32)
            .reshape(cfg.H, cfg.TOPO).T)
    for nm in ("v", "a"):
        for k in ("w1", "b1", "w2", "b2"):
            shared[f"{nm}_{k}"] = np.ascontiguousarray(
                np.asarray(inputs[f"{nm}_{k}"], np.float32))
    in_maps = []
    for c in range(cfg.CORES):
        m = dict(shared)
        m["x_slice"] = x[c * cfg.NPC:(c + 1) * cfg.NPC].copy()
        m["src_idx"] = np.ascontiguousarray(src_w[c])
        m["dst_idx"] = np.ascontiguousarray(dst_w[c])
        m["dstloc"] = np.ascontiguousarray(dl_m[c])
        in_maps.append(m)
    return in_maps


def run(inputs, cfg=CFG, trace=False):
    schedule, src_w, dst_w, dl_m = host_prep(inputs["edge_index"], cfg)
    key = (cfg.N, cfg.E, tuple(schedule))
    if key not in _CACHE:
        _CACHE[key] = build_program(cfg, schedule)
    pr = _CACHE[key]
    in_maps = make_in_maps(inputs, cfg, src_w, dst_w, dl_m)
    res = run_bass_kernel_spmd(pr.nc, in_maps, list(range(cfg.CORES)),
                               trace=trace)
    out = res.results[0]
    return (np.asarray(out["valence"], np.float32),
            np.asarray(out["arousal"], np.float32)), res


def kernel(**inputs):
    (val, aro), _ = run(inputs)
    return (val, aro)



# revision 37
# speedup vs baseline: 1.3473x; 1.3473x over previous
"""GCATopo (2-layer GTAT GNN) Trainium2 kernel, 8-way SPMD — v2.

Strategy (v2 redesign vs v1):
 - Node-major aggregation: per 128-edge tile ONE 512-wide matmul
   (lhsT=St one-hot, rhs=et2-weighted gathered features) accumulates
   [dst, 512] in a single PSUM bank; softmax denominators aggregate in a
   second small matmul. Normalization becomes per-partition scaling.
 - Per-edge dst logits come from a lookup matmul (lhsT=StT, rhs=local
   per-block dst-attn rows) instead of a 256B-per-edge DMA gather.
 - All per-edge elementwise work (logits, leaky-relu, exp, message
   weighting) is batched across a block's ~14 tiles with strided 3D/4D
   APs — a handful of DVE/Act instructions per block instead of ~15 per
   tile.
 - L2's topo output is discarded by the model, so L2 ships only
   [feat 512 | ta 4] and skips the SM stream entirely.
 - All matmul operands bf16 (4x PE rate vs f32); weights are host-folded
   (wl@attB etc.) and host-transposed; x arrives pre-transposed bf16.
 - Biases are folded forward into the next layer's constant rows, so
   drains are pure scaling.
 - L2 prep is fused into the L1 edge-phase block loop (PE prep matmuls
   overlap DVE/DMA edge work).
"""

from contextlib import ExitStack

import ml_dtypes
import numpy as np

import concourse.bacc as bacc
import concourse.tile as tile
from concourse import mybir
from concourse.masks import make_identity
from concourse.bass_utils import run_bass_kernel_spmd

F32 = mybir.dt.float32
BF16 = mybir.dt.bfloat16
I16 = mybir.dt.int16
AF = mybir.ActivationFunctionType
OP = mybir.AluOpType

P = 128
BF = ml_dtypes.bfloat16


class Cfg:
    def __init__(self, N=20000, E=240000, FIN=576, HID=128, TOPO=15, H=4,
                 CORES=8, NEG=0.2):
        self.N, self.E, self.FIN, self.HID, self.TOPO, self.H = N, E, FIN, HID, TOPO, H
        self.CORES, self.NEG = CORES, NEG
        self.HC = H * HID                      # 512
        self.ROW = self.HC + 128               # gathered row, bf16 (1280B)
        self.NPC = N // CORES                  # nodes per core
        self.NBLK = (self.NPC + P - 1) // P    # dst blocks per core
        # aux slots within the bf16 row (offsets from 0)
        self.C_TOPO = self.HC                  # 512..526: topo (L1)
        self.C_ONE = self.HC + TOPO            # 527: constant 1.0 (L1)
        self.C_AL = self.HC + 16               # 528..531: al (L1)
        self.C_TA = self.HC + 20               # 532..535: ta (L1)
        self.C_TA2 = self.HC                   # 512..515: ta (L2)


CFG = Cfg()
GT_MAX = 8  # max tiles (=128 idxs each) per gather call


def cdiv(a, b):
    return (a + b - 1) // b


def ktiles(F):
    return [(o, min(P, F - o)) for o in range(0, F, P)]


# --------------------------------------------------------------------------
# host-side graph preprocessing (pure indexing on edge_index)
# --------------------------------------------------------------------------

def host_prep(edge_index, cfg):
    N, CORES, NPC, NBLK = cfg.N, cfg.CORES, cfg.NPC, cfg.NBLK
    src = np.asarray(edge_index[0], dtype=np.int64)
    dst = np.asarray(edge_index[1], dtype=np.int64)
    loops = np.arange(N, dtype=np.int64)
    src = np.concatenate([src, loops])
    dst = np.concatenate([dst, loops])
    order = np.argsort(dst, kind="stable")
    s, d = src[order], dst[order]

    core_of = d // NPC
    blk_of = (d % NPC) // P
    counts = np.zeros((CORES, NBLK), dtype=np.int64)
    for c in range(CORES):
        m = core_of == c
        bb = blk_of[m]
        for b in range(NBLK):
            counts[c, b] = int((bb == b).sum())
    schedule = [max(1, cdiv(int(counts[:, b].max()), P)) for b in range(NBLK)]
    offs = np.concatenate([[0], np.cumsum(schedule)]).astype(np.int64)
    ttot = int(offs[-1])

    srcidx = np.zeros((CORES, ttot * P), dtype=np.int16)
    dstloc = np.full((CORES, ttot * P), -1.0, dtype=np.float32)
    for c in range(CORES):
        m = core_of == c
        sc, dc, bc = s[m], d[m], blk_of[m]
        for b in range(NBLK):
            mb = bc == b
            n = int(mb.sum())
            base = int(offs[b]) * P
            srcidx[c, base:base + n] = sc[mb].astype(np.int16)
            dstloc[c, base:base + n] = (dc[mb] - (c * NPC + b * P)).astype(np.float32)

    # wrap for dma_gather: index i lives at [i % 16, i // 16]; the 16-row
    # block is replicated 8x along partitions (one stripe per gpsimd core)
    src_w = [np.tile(srcidx[c].reshape(-1, 16).T, (8, 1)).copy()
             for c in range(CORES)]
    # dl column view: edge j -> tile j//128, partition j%128
    dl_col = [dstloc[c].reshape(ttot, P).T.astype(BF).copy()
              for c in range(CORES)]
    # dl row view: flat edge-slot order (for partition_broadcast)
    dl_rows = [dstloc[c].astype(BF).reshape(1, ttot * P).copy()
               for c in range(CORES)]
    return schedule, src_w, dl_col, dl_rows


def host_weights(inputs, cfg):
    """All small-weight folding in f32 numpy, shipped as bf16."""
    H, C, TOPO, HC = cfg.H, cfg.HID, cfg.TOPO, cfg.HC
    f = lambda k: np.asarray(inputs[k], np.float32)

    def attB(att):  # [1,H,C] -> block-diag [H*C, H]
        out = np.zeros((H * C, H), np.float32)
        a = np.asarray(att, np.float32).reshape(H, C)
        for h in range(H):
            out[h * C:(h + 1) * C, h] = a[h]
        return out

    w = {}
    # topo extractor
    w["tw1"] = f("te_w1")                      # [576,128]
    w["tb1"] = f("te_b1").reshape(1, -1)
    w["tw2"] = f("te_w2")                      # [128,15]
    w["tb2"] = f("te_b2").reshape(1, -1)
    # layer 1
    aB1 = attB(inputs["l1_att"])
    w["wl1"] = f("l1_wl")                      # [576,512]
    w["bl1"] = f("l1_bl").reshape(1, -1)
    w["A1"] = np.concatenate([f("l1_wl") @ aB1, f("l1_wr") @ aB1], 1)  # [576,8]
    w["bA1"] = np.concatenate([f("l1_bl") @ aB1, f("l1_br") @ aB1]).reshape(1, -1)
    w["att2T1"] = f("l1_att2").reshape(H, TOPO).T      # [15,4]
    # layer 2 (input h1 = agg1_norm, l1_bias folded here)
    b1 = f("l1_bias")
    w["wl2"] = f("l2_wl")                      # [512,512]
    w["bl2"] = (b1 @ f("l2_wl") + f("l2_bl")).reshape(1, -1)
    w["att2T2"] = f("l2_att2").reshape(H, TOPO).T      # [15,4]
    # topo1 input to L2 = topo1_raw + l1_bias2; edge logit gets the const
    # twice (src+dst) -> fold 2*(b2@att2) into the dst-side rows only
    w["ta2c"] = (2.0 * (f("l1_bias2") @ w["att2T2"])).reshape(1, -1)   # [1,4]
    # heads (l2_bias folded into first-layer bias)
    b2f = f("l2_bias")
    for nm in ("v", "a"):
        w[f"{nm}w1"] = f(f"{nm}_w1")           # [512,128]
        w[f"{nm}b1"] = (f(f"{nm}_b1") + b2f @ f(f"{nm}_w1")).reshape(1, -1)
        w[f"{nm}w2"] = f(f"{nm}_w2")           # [128,1]
        w[f"{nm}b2"] = f(f"{nm}_b2").reshape(1, 1)
    # att2T2 flattened (h,j) row for the drain's ta2 reduce + const
    w["att2f"] = w["att2T2"].T.reshape(1, -1)  # [1,60] (h-major)
    return {k: v.astype(BF) for k, v in w.items()}


# --------------------------------------------------------------------------
# program builder
# --------------------------------------------------------------------------

class Prog:
    pass


def build_program(cfg, schedule, debug=False):
    es = ExitStack()
    nc = bacc.Bacc("TRN2", target_bir_lowering=False, debug=False,
                   num_devices=cfg.CORES)
    pr = Prog()
    pr.nc = nc
    N, FIN, HID, TOPO, H, HC, ROW, NPC, NBLK = (
        cfg.N, cfg.FIN, cfg.HID, cfg.TOPO, cfg.H, cfg.HC, cfg.ROW, cfg.NPC,
        cfg.NBLK)
    TTOT = sum(schedule)
    W16 = TTOT * P // 16
    groups = [list(range(cfg.CORES))]
    blocks = ktiles(NPC)
    fkt = ktiles(FIN)
    ckt = ktiles(HC)
    offs = np.concatenate([[0], np.cumsum(schedule)]).astype(int)

    def din(name, shape, dtype=BF16):
        return nc.dram_tensor(name, list(shape), dtype, kind="ExternalInput")

    # ---- external inputs ----
    xT = din("xT_slice", (FIN, NPC))
    wnames = [("tw1", (FIN, HID)), ("tb1", (1, HID)), ("tw2", (HID, TOPO)),
              ("tb2", (1, TOPO)), ("wl1", (FIN, HC)), ("bl1", (1, HC)),
              ("A1", (FIN, 2 * H)), ("bA1", (1, 2 * H)), ("att2T1", (TOPO, H)),
              ("wl2", (HC, HC)), ("bl2", (1, HC)), ("att2T2", (TOPO, H)),
              ("ta2c", (1, H)), ("att2f", (1, H * TOPO)),
              ("vw1", (HC, HID)), ("vb1", (1, HID)), ("vw2", (HID, 1)),
              ("vb2", (1, 1)),
              ("aw1", (HC, HID)), ("ab1", (1, HID)), ("aw2", (HID, 1)),
              ("ab2", (1, 1))]
    W = {nm: din(nm, sh) for nm, sh in wnames}
    src_i = din("src_idx", (P, W16), I16)
    dlc_i = din("dl_col", (P, TTOT))
    dlr_i = din("dl_rows", (1, TTOT * P))

    # ---- outputs ----
    val_o = nc.dram_tensor("valence", [1, 1], F32, kind="ExternalOutput")
    aro_o = nc.dram_tensor("arousal", [1, 1], F32, kind="ExternalOutput")
    dbg = {}
    if debug:
        for nm, sh in [("dbg_h1", (P, HC)), ("dbg_tt", (P, TOPO + H)),
                       ("dbg_psm1", (P, 68)), ("dbg_h2", (P, HC)),
                       ("dbg_pool", (P, H)), ("dbg_aux", (P, 24)),
                       ("dbg_psm2", (P, H)), ("dbg_pd1", (P, 2 * H))]:
            dbg[nm] = nc.dram_tensor(nm, list(sh), F32, kind="ExternalOutput")

    # ---- internal DRAM ----
    ext_sl = [nc.dram_tensor(f"ext_slice{L}", [NPC, ROW], BF16)
              for L in (1, 2)]
    ext_fl = [nc.dram_tensor(f"ext_full{L}", [N, ROW], BF16,
                             addr_space="Shared") for L in (1, 2)]
    pool_in = nc.dram_tensor("pool_in", [1, HC], F32)
    pool_out = nc.dram_tensor("pool_out", [1, HC], F32, addr_space="Shared")

    with tile.TileContext(nc) as tc:
        # ================= static SBUF =================
        ident = nc.alloc_sbuf_tensor("ident", [P, P], F32).ap()
        make_identity(nc, ident)
        iota_i = nc.alloc_sbuf_tensor("iota_i", [P, P], mybir.dt.int32).ap()
        nc.gpsimd.iota(iota_i, pattern=[[1, P]], base=0, channel_multiplier=0)
        iota_row = nc.alloc_sbuf_tensor("iota_row", [P, P], BF16).ap()
        nc.vector.tensor_copy(iota_row, iota_i)
        iota_ci = nc.alloc_sbuf_tensor("iota_ci", [P, 1], mybir.dt.int32).ap()
        nc.gpsimd.iota(iota_ci, pattern=[[1, 1]], base=0, channel_multiplier=1)
        iota_col = nc.alloc_sbuf_tensor("iota_col", [P, 1], BF16).ap()
        nc.vector.tensor_copy(iota_col, iota_ci)
        ones_row = nc.alloc_sbuf_tensor("ones_row", [1, NPC], BF16).ap()
        nc.gpsimd.memset(ones_row, 1.0)
        ones_col = nc.alloc_sbuf_tensor("ones_col", [P, 1], BF16).ap()
        nc.gpsimd.memset(ones_col, 1.0)
        eps_col = nc.alloc_sbuf_tensor("eps_col", [P, 1], F32).ap()
        nc.gpsimd.memset(eps_col, 1e-30)

        src_sb = nc.alloc_sbuf_tensor("src_sb", [P, W16], I16).ap()
        dlc_sb = nc.alloc_sbuf_tensor("dlc_sb", [P, TTOT], BF16).ap()
        nc.sync.dma_start(src_sb, src_i[:, :])
        nc.sync.dma_start(dlc_sb, dlc_i[:, :])

        # resident activations / weights
        xT_sb = [nc.alloc_sbuf_tensor(f"xT{i}", [P, NPC], BF16).ap()
                 for i in range(len(fkt))]
        for i, (fo, fk) in enumerate(fkt):
            nc.sync.dma_start(xT_sb[i][:fk, :], xT[fo:fo + fk, :])
        hfmT = [nc.alloc_sbuf_tensor(f"hfmT{i}", [P, NPC], BF16).ap()
                for i in range(len(ckt))]
        topoT0 = nc.alloc_sbuf_tensor("topoT0", [TOPO, NPC], BF16).ap()
        datt1 = nc.alloc_sbuf_tensor("datt1", [P, NBLK * 2 * H], BF16).ap()
        datt2 = nc.alloc_sbuf_tensor("datt2", [P, NBLK * H], BF16).ap()
        nc.vector.memset(datt1, 0.0)   # rows past a partial block stay 0
        nc.vector.memset(datt2, 0.0)

        wsb = {}
        for nm, sh in wnames:
            if sh[0] <= P:
                wsb[nm] = nc.alloc_sbuf_tensor(f"w_{nm}", list(sh), BF16).ap()
                nc.sync.dma_start(wsb[nm], W[nm][:, :])
            else:  # k-tiled along the first (contraction) dim
                tiles = []
                for i, (fo, fk) in enumerate(ktiles(sh[0])):
                    t = nc.alloc_sbuf_tensor(f"w_{nm}{i}", [fk, sh[1]],
                                             BF16).ap()
                    nc.sync.dma_start(t, W[nm][fo:fo + fk, :])
                    tiles.append(t)
                wsb[nm] = tiles
        # att2f / ta2c broadcast to all partitions
        att2bc = nc.alloc_sbuf_tensor("att2bc", [P, H * TOPO], BF16).ap()
        nc.gpsimd.partition_broadcast(att2bc, wsb["att2f"][0:1, :])
        ta2cbc = nc.alloc_sbuf_tensor("ta2cbc", [P, H], BF16).ap()
        nc.gpsimd.partition_broadcast(ta2cbc, wsb["ta2c"][0:1, :])
        ident_bf = nc.alloc_sbuf_tensor("ident_bf", [P, P], BF16).ap()
        nc.vector.tensor_copy(ident_bf, ident)

        # ================= phase A: topo MLP + L1 prep =================
        with tc.tile_pool(name="ppA", bufs=1, space="PSUM") as ppA, \
             tc.tile_pool(name="ppA2", bufs=2, space="PSUM") as ppA2, \
             tc.tile_pool(name="cpA", bufs=3) as cpA, \
             tc.tile_pool(name="spA", bufs=2) as spA:
            # --- topo extractor MLP (feat-major: out rows = hid/topo) ---
            NG = 512
            for go in range(0, NPC, NG):
                gs = min(NG, NPC - go)
                ph = ppA.tile([P, NG], F32, tag="ph", name="ph", space="PSUM")
                for i, (fo, fk) in enumerate(fkt):
                    nc.tensor.matmul(ph[:, :gs], lhsT=wsb["tw1"][i][:fk, :],
                                     rhs=xT_sb[i][:fk, go:go + gs],
                                     start=i == 0, stop=False,
                                     skip_group_check=True)
                nc.tensor.matmul(ph[:, :gs], lhsT=wsb["tb1"][:, :],
                                 rhs=ones_row[:, go:go + gs], start=False,
                                 stop=True, skip_group_check=True)
                t_hid = spA.tile([P, NG], BF16, tag="t_hid", name="t_hid")
                nc.scalar.activation(t_hid[:, :gs], ph[:, :gs], AF.Relu)
                pt = ppA.tile([TOPO, NG], F32, tag="pt", name="pt", space="PSUM")
                nc.tensor.matmul(pt[:, :gs], lhsT=wsb["tw2"][:, :],
                                 rhs=t_hid[:, :gs], start=True, stop=False,
                                 skip_group_check=True)
                nc.tensor.matmul(pt[:, :gs], lhsT=wsb["tb2"][:, :],
                                 rhs=ones_row[:, go:go + gs], start=False,
                                 stop=True, skip_group_check=True)
                nc.vector.tensor_copy(topoT0[:, go:go + gs], pt[:, :gs])

            # --- L1 prep per block ---
            for bi, (bo, bs) in enumerate(blocks):
                pm = ppA2.tile([P, HC], F32, tag="pm", name="pm", space="PSUM")
                pa = ppA.tile([P, 2 * H], F32, tag="pa", name="pa", space="PSUM")
                for i, (fo, fk) in enumerate(fkt):
                    nc.tensor.matmul(pm[:bs, :], lhsT=xT_sb[i][:fk, bo:bo + bs],
                                     rhs=wsb["wl1"][i][:fk, :],
                                     start=i == 0, stop=False,
                                     skip_group_check=True)
                    nc.tensor.matmul(pa[:bs, :], lhsT=xT_sb[i][:fk, bo:bo + bs],
                                     rhs=wsb["A1"][i][:fk, :],
                                     start=i == 0, stop=False,
                                     skip_group_check=True)
                nc.tensor.matmul(pm[:bs, :], lhsT=ones_row[:, bo:bo + bs],
                                 rhs=wsb["bl1"][:, :], start=False, stop=True,
                                 skip_group_check=True)
                nc.tensor.matmul(pa[:bs, :], lhsT=ones_row[:, bo:bo + bs],
                                 rhs=wsb["bA1"][:, :], start=False, stop=True,
                                 skip_group_check=True)
                pta = ppA.tile([P, H], F32, tag="pta", name="pta", space="PSUM")
                nc.tensor.matmul(pta[:bs, :], lhsT=topoT0[:, bo:bo + bs],
                                 rhs=wsb["att2T1"][:, :], start=True,
                                 stop=True, skip_group_check=True)
                ptt = ppA.tile([P, TOPO], BF16, tag="ptt", name="ptt",
                               space="PSUM")
                nc.tensor.transpose(ptt[:bs, :TOPO],
                                    topoT0[:, bo:bo + bs],
                                    ident_bf[:TOPO, :TOPO])
                ext = cpA.tile([P, ROW], BF16, tag="ext", name="ext")
                nc.scalar.copy(ext[:bs, 0:HC], pm[:bs, :])
                nc.scalar.copy(ext[:bs, cfg.C_TOPO:cfg.C_TOPO + TOPO],
                               ptt[:bs, :TOPO])
                nc.vector.memset(ext[:bs, cfg.C_ONE:cfg.C_ONE + 1], 1.0)
                nc.scalar.copy(ext[:bs, cfg.C_AL:cfg.C_AL + H], pa[:bs, 0:H])
                nc.scalar.copy(ext[:bs, cfg.C_TA:cfg.C_TA + H], pta[:bs, :])
                nc.sync.dma_start(ext_sl[0][bo:bo + bs, :], ext[:bs, :])
                if debug and bi == 0:
                    da = cpA.tile([P, 24], F32, tag="dbga", name="dbga")
                    nc.vector.tensor_copy(da[:, :], ext[:, HC:HC + 24])
                    nc.sync.dma_start(dbg["dbg_aux"][:, :], da[:, :])
                # dst-side rows: [ar | ta]
                nc.vector.tensor_copy(datt1[:bs, bi * 2 * H:bi * 2 * H + H],
                                      pa[:bs, H:2 * H])
                nc.vector.tensor_copy(
                    datt1[:bs, bi * 2 * H + H:(bi + 1) * 2 * H], pta[:bs, :])
            nc.gpsimd.collective_compute(
                "AllGather", OP.bypass, replica_groups=groups,
                ins=[ext_sl[0][:, :]], outs=[ext_fl[0][:, :]])

        # ================= edge phase (shared emitter) =================
        TMAX = max(schedule)

        def emit_edge(L, gp, sp, pp, pp2):
            AUXW = 2 * H if L == 1 else H      # lg width per tile
            AUXO = cfg.C_AL if L == 1 else cfg.C_TA2
            for bi, (bo, bs) in enumerate(blocks):
                Tb = schedule[bi]
                base = int(offs[bi])
                TW = Tb * P
                # ---- gathers ----
                G = gp.tile([P, TMAX * ROW], BF16, tag="G", name="G")
                for go in range(0, Tb, GT_MAX):
                    gn = min(GT_MAX, Tb - go)
                    c0 = (base + go) * 8
                    nc.gpsimd.dma_gather(
                        G[:, go * ROW:(go + gn) * ROW].rearrange(
                            "p (t e) -> p t e", e=ROW),
                        ext_fl[L - 1][:, :], src_sb[:, c0:c0 + 8 * gn],
                        num_idxs=P * gn, num_idxs_reg=P * gn, elem_size=ROW,
                        queue_num=0)
                # ---- St / StT ----
                St = sp.tile([P, TMAX * P], BF16, tag="St", name="St")
                nc.vector.tensor_tensor(
                    St[:, 0:TW].rearrange("p (t d) -> p t d", d=P),
                    iota_row[:, :].unsqueeze(1).to_broadcast((P, Tb, P)),
                    dlc_sb[:, base:base + Tb].unsqueeze(2).to_broadcast(
                        (P, Tb, P)),
                    OP.is_equal)
                dlr_st = sp.tile([1, TMAX * P], BF16, tag="dlr", name="dlr")
                nc.sync.dma_start(dlr_st[:, 0:TW],
                                  dlr_i[:, base * P:base * P + TW])
                dlR = sp.tile([P, TMAX * P], BF16, tag="dlR", name="dlR")
                nc.gpsimd.partition_broadcast(dlR[:, 0:TW], dlr_st[0:1, 0:TW])
                StT = sp.tile([P, TMAX * P], BF16, tag="StT", name="StT")
                nc.vector.tensor_tensor(
                    StT[:, 0:TW], iota_col[:, :].to_broadcast((P, TW)),
                    dlR[:, 0:TW], OP.is_equal)
                # ---- dst-logit lookup ----
                pD = pp.tile([P, TMAX * AUXW], F32, tag="pD", name="pD",
                             space="PSUM")
                dsl = (datt1[:, bi * 2 * H:(bi + 1) * 2 * H] if L == 1
                       else datt2[:, bi * H:(bi + 1) * H])
                for t in range(Tb):
                    nc.tensor.matmul(pD[:, t * AUXW:(t + 1) * AUXW],
                                     lhsT=StT[:, t * P:(t + 1) * P],
                                     rhs=dsl, start=True, stop=True,
                                     skip_group_check=True)
                # ---- batched logits ----
                Gv = G[:, 0:Tb * ROW].rearrange("p (t e) -> p t e", e=ROW)
                lg = sp.tile([P, TMAX * AUXW], F32, tag="lg", name="lg")
                nc.vector.tensor_tensor(
                    lg[:, 0:Tb * AUXW].rearrange("p (t c) -> p t c", c=AUXW),
                    Gv[:, :, AUXO:AUXO + AUXW],
                    pD[:, 0:Tb * AUXW].rearrange("p (t c) -> p t c", c=AUXW),
                    OP.add)
                lr = sp.tile([P, TMAX * AUXW], F32, tag="lr", name="lr")
                nc.vector.scalar_tensor_tensor(
                    lr[:, 0:Tb * AUXW], lg[:, 0:Tb * AUXW], cfg.NEG,
                    lg[:, 0:Tb * AUXW], OP.mult, OP.max)
                et = sp.tile([P, TMAX * AUXW], BF16, tag="et", name="et")
                nc.scalar.activation(et[:, 0:Tb * AUXW], lr[:, 0:Tb * AUXW],
                                     AF.Exp)
                etv = et[:, 0:Tb * AUXW].rearrange("p (t c) -> p t c", c=AUXW)
                # ---- weighted messages ----
                Gp = gp.tile([P, TMAX * HC], BF16, tag="Gp", name="Gp")
                e2off = H if L == 1 else 0
                nc.vector.tensor_tensor(
                    Gp[:, 0:Tb * HC].rearrange("p (t h c) -> p t h c",
                                               h=H, c=HID),
                    Gv[:, :, 0:HC].rearrange("p t (h c) -> p t h c", c=HID),
                    etv[:, :, e2off:e2off + H].unsqueeze(3).to_broadcast(
                        (P, Tb, H, HID)),
                    OP.mult)
                if L == 1:
                    SMW = 16 * H + H
                    SMe = sp.tile([P, TMAX * SMW], BF16, tag="SMe", name="SMe")
                    SMv = SMe[:, 0:Tb * SMW].rearrange("p (t c) -> p t c",
                                                       c=SMW)
                    nc.vector.tensor_tensor(
                        SMv[:, :, 0:16 * H].rearrange(
                            "p t (h j) -> p t h j", j=16),
                        Gv[:, :, HC:HC + 16].unsqueeze(2).to_broadcast(
                            (P, Tb, H, 16)),
                        etv[:, :, 0:H].unsqueeze(3).to_broadcast(
                            (P, Tb, H, 16)),
                        OP.mult)
                    nc.scalar.copy(SMv[:, :, 16 * H:SMW],
                                   etv[:, :, H:2 * H])
                else:
                    SMW = H
                    SMe = et
                # ---- aggregation matmuls ----
                pf = pp2.tile([P, HC], F32, tag="pf", name="pf", space="PSUM")
                psm = pp.tile([P, SMW], F32, tag="psm", name="psm",
                              space="PSUM")
                for t in range(Tb):
                    st0, sp1 = t == 0, t == Tb - 1
                    nc.tensor.matmul(pf[:, :], lhsT=St[:, t * P:(t + 1) * P],
                                     rhs=Gp[:, t * HC:(t + 1) * HC],
                                     start=st0, stop=sp1,
                                     skip_group_check=True)
                    nc.tensor.matmul(psm[:, :],
                                     lhsT=St[:, t * P:(t + 1) * P],
                                     rhs=SMe[:, t * SMW:(t + 1) * SMW],
                                     start=st0, stop=sp1,
                                     skip_group_check=True)
                if debug and bi == 0:
                    dt = sp.tile([P, 68], F32, tag="dbgp", name="dbgp")
                    nc.vector.tensor_copy(dt[:, 0:SMW], psm[:, :])
                    nc.sync.dma_start(
                        dbg["dbg_psm1" if L == 1 else "dbg_psm2"][:, 0:SMW],
                        dt[:, 0:SMW])
                    dp = sp.tile([P, 2 * H], F32, tag="dbgd", name="dbgd")
                    nc.vector.tensor_copy(dp[:, 0:AUXW], pD[:, 0:AUXW])
                    if L == 1:
                        nc.sync.dma_start(dbg["dbg_pd1"][:, 0:AUXW],
                                          dp[:, 0:AUXW])
                # ---- drain ----
                if L == 1:
                    drain1(bi, bo, bs, pf, psm, sp, pp, pp2)
                else:
                    drain2(bi, bo, bs, pf, psm, sp, pp)

        # ---- L1 drain + fused L2 prep ----
        def drain1(bi, bo, bs, pf, psm, sp, pp, pp2):
            # rec2 = 1/sum(e2), rec1' = 1/(H*sum(e1))
            den = sp.tile([P, 2 * H], F32, tag="den", name="den")
            nc.vector.tensor_scalar(
                den[:, 0:H].unsqueeze(2),
                psm[:, 0:16 * H].rearrange("p (h j) -> p h j", j=16)[
                    :, :, 15:16],
                float(H), eps_col[:, 0:1], OP.mult, OP.max)
            nc.vector.tensor_tensor(den[:, H:2 * H], psm[:, 16 * H:16 * H + H],
                                    eps_col[:, 0:1].to_broadcast((P, H)),
                                    OP.max)
            rec = sp.tile([P, 2 * H], F32, tag="rec", name="rec")
            nc.vector.reciprocal(rec[:, :], den[:, :])
            # h1 = agg_feat * rec2 (node-major, bf16; per-head scale on Act)
            h1 = sp.tile([P, HC], BF16, tag="h1", name="h1")
            for h in range(H):
                nc.scalar.activation(h1[:, h * HID:(h + 1) * HID],
                                     pf[:, h * HID:(h + 1) * HID], AF.Copy,
                                     scale=rec[:, H + h:H + h + 1])
            # topo1_raw = sum_h agg_topo_h * rec1'   [d, 15]
            tp = sp.tile([P, TOPO * H], F32, tag="tp", name="tp")
            nc.vector.tensor_tensor(
                tp[:, :].rearrange("p (j h) -> p j h", h=H),
                psm[:, 0:16 * H].rearrange("p (h j) -> p h j", j=16)[
                    :, :, 0:TOPO].transpose([0, 2, 1]),
                rec[:, 0:H].unsqueeze(1).to_broadcast((P, TOPO, H)),
                OP.mult)
            t1 = sp.tile([P, TOPO], F32, tag="t1", name="t1")
            nc.vector.tensor_reduce(
                t1[:, :], tp[:, :].rearrange("p (j h) -> p j h", h=H),
                mybir.AxisListType.X, OP.add)
            # ta2 = topo1_raw @ att2T2 (per-node, via DVE reduce)
            tq = sp.tile([P, H * TOPO], F32, tag="tq", name="tq")
            nc.vector.tensor_tensor(
                tq[:, :].rearrange("p (h j) -> p h j", j=TOPO),
                t1[:, :].unsqueeze(1).to_broadcast((P, H, TOPO)),
                att2bc[:, :].rearrange("p (h j) -> p h j", j=TOPO),
                OP.mult)
            ta2 = sp.tile([P, H], F32, tag="ta2", name="ta2")
            nc.vector.tensor_reduce(
                ta2[:, :], tq[:, :].rearrange("p (h j) -> p h j", j=TOPO),
                mybir.AxisListType.X, OP.add)
            # dst rows for L2: ta2 + 2*(b2@att2)
            nc.vector.tensor_tensor(datt2[:bs, bi * H:(bi + 1) * H],
                                    ta2[:bs, :],
                                    ta2cbc[:bs, :], OP.add)
            if debug and bi == 0:
                dh = sp.tile([P, HC], F32, tag="dbgh", name="dbgh")
                nc.vector.tensor_copy(dh[:, :], h1[:, :])
                nc.sync.dma_start(dbg["dbg_h1"][:, :], dh[:, :])
                dtt = sp.tile([P, TOPO + H], F32, tag="dbgt", name="dbgt")
                nc.vector.tensor_copy(dtt[:, 0:TOPO], t1[:, :])
                nc.vector.tensor_copy(dtt[:, TOPO:TOPO + H], ta2[:, :])
                nc.sync.dma_start(dbg["dbg_tt"][:, :], dtt[:, :])
            # transpose h1 -> hfmT tiles
            for ci, (co, ck) in enumerate(ckt):
                ptr = pp.tile([P, P], BF16, tag="ptr", name="ptr",
                              space="PSUM")
                nc.tensor.transpose(ptr[:ck, :bs], h1[:bs, co:co + ck],
                                    ident_bf[:bs, :bs])
                nc.scalar.copy(hfmT[ci][:ck, bo:bo + bs], ptr[:ck, :bs])
            # ---- fused L2 prep for this block ----
            pm2 = pp2.tile([P, HC], F32, tag="pm2", name="pm2", space="PSUM")
            for ci, (co, ck) in enumerate(ckt):
                nc.tensor.matmul(pm2[:bs, :], lhsT=hfmT[ci][:ck, bo:bo + bs],
                                 rhs=wsb["wl2"][ci][:ck, :],
                                 start=ci == 0, stop=False,
                                 skip_group_check=True)
            nc.tensor.matmul(pm2[:bs, :], lhsT=ones_row[:, bo:bo + bs],
                             rhs=wsb["bl2"][:, :], start=False, stop=True,
                             skip_group_check=True)
            ext = sp.tile([P, ROW], BF16, tag="ext2", name="ext2")
            nc.scalar.copy(ext[:bs, 0:HC], pm2[:bs, :])
            nc.scalar.copy(ext[:bs, cfg.C_TA2:cfg.C_TA2 + H], ta2[:bs, :])
            nc.sync.dma_start(ext_sl[1][bo:bo + bs, :], ext[:bs, :])

        # ---- L2 drain: normalize + pooled partial ----
        def drain2(bi, bo, bs, pf, psm, sp, pp):
            den = sp.tile([P, H], F32, tag="den2", name="den2")
            nc.vector.tensor_tensor(den[:, :], psm[:, 0:H],
                                    eps_col[:, 0:1].to_broadcast((P, H)),
                                    OP.max)
            rec = sp.tile([P, H], F32, tag="rec2", name="rec2")
            nc.vector.reciprocal(rec[:, :], den[:, :])
            h2 = sp.tile([P, HC], BF16, tag="h2", name="h2")
            for h in range(H):
                nc.scalar.activation(h2[:, h * HID:(h + 1) * HID],
                                     pf[:, h * HID:(h + 1) * HID], AF.Copy,
                                     scale=rec[:, h:h + 1])
            if debug and bi == 0:
                dh = sp.tile([P, HC], F32, tag="dbgh2", name="dbgh2")
                nc.vector.tensor_copy(dh[:, :], h2[:, :])
                nc.sync.dma_start(dbg["dbg_h2"][:, :], dh[:, :])
            nc.tensor.matmul(pr.pool_ps[:, :], lhsT=ones_col[:bs, 0:1],
                             rhs=h2[:bs, :], start=bi == 0,
                             stop=bi == len(blocks) - 1,
                             skip_group_check=True)

        # ================= phase B/C: L1 edges (+L2 prep) =================
        # PSUM banks: pf 2 + pm2 2 + pD/psm/ptr 1 each = 7 of 8
        with tc.tile_pool(name="gpB", bufs=2) as gpB, \
             tc.tile_pool(name="spB", bufs=2) as spB, \
             tc.tile_pool(name="ppB", bufs=1, space="PSUM") as ppB, \
             tc.tile_pool(name="ppB2", bufs=2, space="PSUM") as ppB2:
            emit_edge(1, gpB, spB, ppB, ppB2)
            nc.gpsimd.collective_compute(
                "AllGather", OP.bypass, replica_groups=groups,
                ins=[ext_sl[1][:, :]], outs=[ext_fl[1][:, :]])

        # ================= phase D: L2 edges =================
        with tc.tile_pool(name="gpD", bufs=2) as gpD, \
             tc.tile_pool(name="spD", bufs=2) as spD, \
             tc.tile_pool(name="ppD", bufs=1, space="PSUM") as ppD, \
             tc.tile_pool(name="ppD2", bufs=2, space="PSUM") as ppD2, \
             tc.tile_pool(name="plD", bufs=1, space="PSUM") as plD:
            pr.pool_ps = plD.tile([1, HC], F32, tag="pool", name="pool",
                                  space="PSUM", bufs=1)
            emit_edge(2, gpD, spD, ppD, ppD2)

            # ---- pool + heads ----
            pooled = spD.tile([1, HC], F32, tag="pooled", name="pooled")
            nc.vector.tensor_copy(pooled[:, :], pr.pool_ps[:, :])
            nc.sync.dma_start(pool_in[:, :], pooled[:, :])
            nc.gpsimd.collective_compute(
                "AllReduce", OP.add, replica_groups=groups,
                ins=[pool_in[:, :]], outs=[pool_out[:, :]])
            # load back column-major: pmean_cols[c, h] = pool_out[h*HID+c]
            pooled2 = spD.tile([P, H], F32, tag="pooled2", name="pooled2")
            with nc.allow_non_contiguous_dma("pool row -> col-major reload"):
                nc.sync.dma_start(
                    pooled2[:, :],
                    pool_out[:, :].rearrange("o (h c) -> (o c) h", c=HID))
            if debug:
                nc.sync.dma_start(dbg["dbg_pool"][:, :], pooled2[:, :])
            pmean = spD.tile([P, H], BF16, tag="pmean", name="pmean")
            nc.vector.tensor_scalar(pmean[:, :], pooled2[:, :], 1.0 / N,
                                    None, OP.mult)
            for nm, out_t in (("v", val_o), ("a", aro_o)):
                pm = ppD.tile([P, 1], F32, tag="mlp", name="mlp", space="PSUM")
                for ki in range(H):
                    nc.tensor.matmul(pm[:, :], lhsT=wsb[f"{nm}w1"][ki][:, :],
                                     rhs=pmean[:, ki:ki + 1], start=ki == 0,
                                     stop=False, skip_group_check=True)
                nc.tensor.matmul(pm[:, :], lhsT=wsb[f"{nm}b1"][:, :],
                                 rhs=ones_col[0:1, :], start=False, stop=True,
                                 skip_group_check=True)
                hv = spD.tile([P, 1], BF16, tag=f"{nm}hv", name=f"{nm}hv")
                nc.scalar.activation(hv[:, :], pm[:, :], AF.Relu)
                po = ppD.tile([1, 1], F32, tag="mlpo", name="mlpo",
                              space="PSUM")
                nc.tensor.matmul(po[:, :], lhsT=hv[:, :],
                                 rhs=wsb[f"{nm}w2"][:, :], start=True,
                                 stop=False, skip_group_check=True)
                nc.tensor.matmul(po[:, :], lhsT=wsb[f"{nm}b2"][:, :],
                                 rhs=ones_col[0:1, :], start=False, stop=True,
                                 skip_group_check=True)
                ov = spD.tile([1, 1], F32, tag=f"{nm}ov", name=f"{nm}ov")
                nc.vector.tensor_copy(ov[:, :], po[:, :])
                nc.sync.dma_start(out_t[:, :], ov[:, :])

    nc.compile()
    es.close()
    return pr


# --------------------------------------------------------------------------
# entry point
# --------------------------------------------------------------------------

_CACHE = {}


def make_in_maps(inputs, cfg, src_w, dl_col, dl_rows):
    x = np.asarray(inputs["x"], dtype=np.float32)
    shared = host_weights(inputs, cfg)
    in_maps = []
    for c in range(cfg.CORES):
        m = dict(shared)
        m["xT_slice"] = np.ascontiguousarray(
            x[c * cfg.NPC:(c + 1) * cfg.NPC].T.astype(BF))
        m["src_idx"] = np.ascontiguousarray(src_w[c])
        m["dl_col"] = np.ascontiguousarray(dl_col[c])
        m["dl_rows"] = np.ascontiguousarray(dl_rows[c])
        in_maps.append(m)
    return in_maps


def run(inputs, cfg=CFG, trace=False):
    schedule, src_w, dl_col, dl_rows = host_prep(inputs["edge_index"], cfg)
    key = (cfg.N, cfg.E, tuple(schedule))
    if key not in _CACHE:
        _CACHE[key] = build_program(cfg, schedule)
    pr = _CACHE[key]
    in_maps = make_in_maps(inputs, cfg, src_w, dl_col, dl_rows)
    res = run_bass_kernel_spmd(pr.nc, in_maps, list(range(cfg.CORES)),
                               trace=trace)
    out = res.results[0]
    return (np.asarray(out["valence"], np.float32),
            np.asarray(out["arousal"], np.float32)), res


def kernel(**inputs):
    (val, aro), _ = run(inputs)
    return (val, aro)


# revision 52
# speedup vs baseline: 1.5214x; 1.1292x over previous
"""GCATopo (2-layer GTAT GNN) Trainium2 kernel, 8-way SPMD — v2.

Strategy (v2 redesign vs v1):
 - Node-major aggregation: per 128-edge tile ONE 512-wide matmul
   (lhsT=St one-hot, rhs=et2-weighted gathered features) accumulates
   [dst, 512] in a single PSUM bank; softmax denominators aggregate in a
   second small matmul. Normalization becomes per-partition scaling.
 - Per-edge dst logits come from a lookup matmul (lhsT=StT, rhs=local
   per-block dst-attn rows) instead of a 256B-per-edge DMA gather.
 - All per-edge elementwise work (logits, leaky-relu, exp, message
   weighting) is batched across a block's ~14 tiles with strided 3D/4D
   APs — a handful of DVE/Act instructions per block instead of ~15 per
   tile.
 - L2's topo output is discarded by the model, so L2 ships only
   [feat 512 | ta 4] and skips the SM stream entirely.
 - All matmul operands bf16 (4x PE rate vs f32); weights are host-folded
   (wl@attB etc.) and host-transposed; x arrives pre-transposed bf16.
 - Biases are folded forward into the next layer's constant rows, so
   drains are pure scaling.
 - L2 prep is fused into the L1 edge-phase block loop (PE prep matmuls
   overlap DVE/DMA edge work).
"""

from contextlib import ExitStack

import ml_dtypes
import numpy as np

import concourse.bacc as bacc
import concourse.tile as tile
from concourse import mybir
from concourse.masks import make_identity
from concourse.bass_utils import run_bass_kernel_spmd

F32 = mybir.dt.float32
BF16 = mybir.dt.bfloat16
F8 = mybir.dt.float8e4
I16 = mybir.dt.int16
AF = mybir.ActivationFunctionType
OP = mybir.AluOpType

P = 128
BF = ml_dtypes.bfloat16


class Cfg:
    def __init__(self, N=20000, E=240000, FIN=576, HID=128, TOPO=15, H=4,
                 CORES=8, NEG=0.2):
        self.N, self.E, self.FIN, self.HID, self.TOPO, self.H = N, E, FIN, HID, TOPO, H
        self.CORES, self.NEG = CORES, NEG
        self.HC = H * HID                      # 512
        self.ROW = 768                         # gathered row: fp8 feat + pad
        self.RB = self.ROW // 2                # bf16 view width (384)
        self.NPC = N // CORES                  # nodes per core
        self.NBLK = (self.NPC + P - 1) // P    # dst blocks per core
        # aux slots within the BF16 VIEW of the row (bf16 element offsets;
        # feat occupies bf16-view [0:256))
        self.C_TOPO = 256                      # 256..270: topo (L1)
        self.C_ONE = 256 + TOPO                # 271: constant 1.0 (L1)
        self.C_AL = 272                        # 272..275: al (L1)
        self.C_TA = 276                        # 276..279: ta (L1)
        self.C_TA2 = 256                       # 256..259: ta (L2)


CFG = Cfg()
GT_MAX = 8  # max tiles (=128 idxs each) per gather call
GP_POOL_FRAC = 3  # tenths of Gp tiles offloaded DVE -> gpsimd


def cdiv(a, b):
    return (a + b - 1) // b


def ktiles(F):
    return [(o, min(P, F - o)) for o in range(0, F, P)]


# --------------------------------------------------------------------------
# host-side graph preprocessing (pure indexing on edge_index)
# --------------------------------------------------------------------------

def host_prep(edge_index, cfg):
    N, CORES, NPC, NBLK = cfg.N, cfg.CORES, cfg.NPC, cfg.NBLK
    src = np.asarray(edge_index[0], dtype=np.int64)
    dst = np.asarray(edge_index[1], dtype=np.int64)
    loops = np.arange(N, dtype=np.int64)
    src = np.concatenate([src, loops])
    dst = np.concatenate([dst, loops])
    order = np.argsort(dst, kind="stable")
    s, d = src[order], dst[order]

    core_of = d // NPC
    blk_of = (d % NPC) // P
    counts = np.zeros((CORES, NBLK), dtype=np.int64)
    for c in range(CORES):
        m = core_of == c
        bb = blk_of[m]
        for b in range(NBLK):
            counts[c, b] = int((bb == b).sum())
    schedule = [max(1, cdiv(int(counts[:, b].max()), P)) for b in range(NBLK)]
    offs = np.concatenate([[0], np.cumsum(schedule)]).astype(np.int64)
    ttot = int(offs[-1])

    srcidx = np.zeros((CORES, ttot * P), dtype=np.int16)
    dstloc = np.full((CORES, ttot * P), -1.0, dtype=np.float32)
    for c in range(CORES):
        m = core_of == c
        sc, dc, bc = s[m], d[m], blk_of[m]
        for b in range(NBLK):
            mb = bc == b
            n = int(mb.sum())
            base = int(offs[b]) * P
            srcidx[c, base:base + n] = sc[mb].astype(np.int16)
            dstloc[c, base:base + n] = (dc[mb] - (c * NPC + b * P)).astype(np.float32)

    # wrap for dma_gather: index i lives at [i % 16, i // 16]; the 16-row
    # block is replicated 8x along partitions (one stripe per gpsimd core)
    src_w = [np.tile(srcidx[c].reshape(-1, 16).T, (8, 1)).copy()
             for c in range(CORES)]
    # host-built one-hot selection tables, per tile [St | StT] (bf16 0/1):
    #   St[e, d] = (dstloc[e] == d), StT = St^T
    rng = np.arange(P, dtype=np.float32)
    stt = []
    for c in range(CORES):
        dl = dstloc[c].reshape(ttot, P)
        St = (dl[:, :, None] == rng[None, None, :])          # [t, e, d]
        tab = np.concatenate([St, St.transpose(0, 2, 1)], 2)  # [t, p, 256]
        stt.append(np.ascontiguousarray(
            tab.transpose(1, 0, 2).reshape(P, ttot * 2 * P).astype(BF)))
    return schedule, src_w, stt


def host_weights(inputs, cfg):
    """All small-weight folding in f32 numpy, shipped as bf16."""
    H, C, TOPO, HC = cfg.H, cfg.HID, cfg.TOPO, cfg.HC
    f = lambda k: np.asarray(inputs[k], np.float32)

    def attB(att):  # [1,H,C] -> block-diag [H*C, H]
        out = np.zeros((H * C, H), np.float32)
        a = np.asarray(att, np.float32).reshape(H, C)
        for h in range(H):
            out[h * C:(h + 1) * C, h] = a[h]
        return out

    w = {}
    # topo extractor
    w["tw1"] = f("te_w1")                      # [576,128]
    w["tb1"] = f("te_b1").reshape(1, -1)
    w["tw2"] = f("te_w2")                      # [128,15]
    w["tb2"] = f("te_b2").reshape(1, -1)
    # layer 1
    aB1 = attB(inputs["l1_att"])
    w["wl1"] = f("l1_wl")                      # [576,512]
    w["bl1"] = f("l1_bl").reshape(1, -1)
    w["A1"] = np.concatenate([f("l1_wl") @ aB1, f("l1_wr") @ aB1], 1)  # [576,8]
    w["bA1"] = np.concatenate([f("l1_bl") @ aB1, f("l1_br") @ aB1]).reshape(1, -1)
    w["att2T1"] = f("l1_att2").reshape(H, TOPO).T      # [15,4]
    # layer 2 (input h1 = agg1_norm, l1_bias folded here)
    b1 = f("l1_bias")
    w["wl2"] = f("l2_wl")                      # [512,512]
    w["bl2"] = (b1 @ f("l2_wl") + f("l2_bl")).reshape(1, -1)
    w["att2T2"] = f("l2_att2").reshape(H, TOPO).T      # [15,4]
    # topo1 input to L2 = topo1_raw + l1_bias2; edge logit gets the const
    # twice (src+dst) -> fold 2*(b2@att2) into the dst-side rows only
    w["ta2c"] = (2.0 * (f("l1_bias2") @ w["att2T2"])).reshape(1, -1)   # [1,4]
    # heads (l2_bias folded into first-layer bias)
    b2f = f("l2_bias")
    for nm in ("v", "a"):
        w[f"{nm}w1"] = f(f"{nm}_w1")           # [512,128]
        w[f"{nm}b1"] = (f(f"{nm}_b1") + b2f @ f(f"{nm}_w1")).reshape(1, -1)
        w[f"{nm}w2"] = f(f"{nm}_w2")           # [128,1]
        w[f"{nm}b2"] = f(f"{nm}_b2").reshape(1, 1)
    # att2T2 flattened (h,j) row for the drain's ta2 reduce + const
    w["att2f"] = w["att2T2"].T.reshape(1, -1)  # [1,60] (h-major)
    return {k: v.astype(BF) for k, v in w.items()}


# --------------------------------------------------------------------------
# program builder
# --------------------------------------------------------------------------

class Prog:
    pass


def build_program(cfg, schedule, debug=False):
    es = ExitStack()
    nc = bacc.Bacc("TRN2", target_bir_lowering=False, debug=False,
                   num_devices=cfg.CORES)
    pr = Prog()
    pr.nc = nc
    N, FIN, HID, TOPO, H, HC, ROW, NPC, NBLK = (
        cfg.N, cfg.FIN, cfg.HID, cfg.TOPO, cfg.H, cfg.HC, cfg.ROW, cfg.NPC,
        cfg.NBLK)
    TTOT = sum(schedule)
    W16 = TTOT * P // 16
    groups = [list(range(cfg.CORES))]
    blocks = ktiles(NPC)
    fkt = ktiles(FIN)
    ckt = ktiles(HC)
    offs = np.concatenate([[0], np.cumsum(schedule)]).astype(int)

    def din(name, shape, dtype=BF16):
        return nc.dram_tensor(name, list(shape), dtype, kind="ExternalInput")

    # ---- external inputs ----
    xT = din("xT_slice", (FIN, NPC))
    wnames = [("tw1", (FIN, HID)), ("tb1", (1, HID)), ("tw2", (HID, TOPO)),
              ("tb2", (1, TOPO)), ("wl1", (FIN, HC)), ("bl1", (1, HC)),
              ("A1", (FIN, 2 * H)), ("bA1", (1, 2 * H)), ("att2T1", (TOPO, H)),
              ("wl2", (HC, HC)), ("bl2", (1, HC)), ("att2T2", (TOPO, H)),
              ("ta2c", (1, H)), ("att2f", (1, H * TOPO)),
              ("vw1", (HC, HID)), ("vb1", (1, HID)), ("vw2", (HID, 1)),
              ("vb2", (1, 1)),
              ("aw1", (HC, HID)), ("ab1", (1, HID)), ("aw2", (HID, 1)),
              ("ab2", (1, 1))]
    W = {nm: din(nm, sh) for nm, sh in wnames}
    src_i = din("src_idx", (P, W16), I16)
    stt_i = din("stt_tab", (P, TTOT * 2 * P))

    # ---- outputs ----
    val_o = nc.dram_tensor("valence", [1, 1], F32, kind="ExternalOutput")
    aro_o = nc.dram_tensor("arousal", [1, 1], F32, kind="ExternalOutput")
    dbg = {}
    if debug:
        for nm, sh in [("dbg_h1", (P, HC)), ("dbg_tt", (P, TOPO + H)),
                       ("dbg_psm1", (P, 68)), ("dbg_h2", (P, HC)),
                       ("dbg_pool", (P, H)), ("dbg_aux", (P, 24)),
                       ("dbg_psm2", (P, H)), ("dbg_pd1", (P, 2 * H))]:
            dbg[nm] = nc.dram_tensor(nm, list(sh), F32, kind="ExternalOutput")

    # ---- internal DRAM ----
    ext_sl = [nc.dram_tensor(f"ext_slice{L}", [NPC, ROW], F8)
              for L in (1, 2)]
    ext_fl = [nc.dram_tensor(f"ext_full{L}", [N, ROW], F8,
                             addr_space="Shared") for L in (1, 2)]
    pool_in = nc.dram_tensor("pool_in", [1, HC], F32)
    pool_out = nc.dram_tensor("pool_out", [1, HC], F32, addr_space="Shared")

    with tile.TileContext(nc) as tc:
        # ================= static SBUF =================
        ident = nc.alloc_sbuf_tensor("ident", [P, P], F32).ap()
        make_identity(nc, ident)
        ones_row = nc.alloc_sbuf_tensor("ones_row", [1, NPC], BF16).ap()
        nc.gpsimd.memset(ones_row, 1.0)
        ones_col = nc.alloc_sbuf_tensor("ones_col", [P, 1], BF16).ap()
        nc.gpsimd.memset(ones_col, 1.0)
        eps_col = nc.alloc_sbuf_tensor("eps_col", [P, 1], F32).ap()
        nc.gpsimd.memset(eps_col, 1e-30)

        src_sb = nc.alloc_sbuf_tensor("src_sb", [P, W16], I16).ap()
        nc.sync.dma_start(src_sb, src_i[:, :])

        # resident activations / weights
        xT_sb = [nc.alloc_sbuf_tensor(f"xT{i}", [P, NPC], BF16).ap()
                 for i in range(len(fkt))]
        for i, (fo, fk) in enumerate(fkt):
            nc.sync.dma_start(xT_sb[i][:fk, :], xT[fo:fo + fk, :])
        hfmT = [nc.alloc_sbuf_tensor(f"hfmT{i}", [P, NPC], BF16).ap()
                for i in range(len(ckt))]
        topoT0 = nc.alloc_sbuf_tensor("topoT0", [TOPO, NPC], BF16).ap()
        datt1 = nc.alloc_sbuf_tensor("datt1", [P, NBLK * 2 * H], BF16).ap()
        datt2 = nc.alloc_sbuf_tensor("datt2", [P, NBLK * H], BF16).ap()
        nc.vector.memset(datt1, 0.0)   # rows past a partial block stay 0
        nc.vector.memset(datt2, 0.0)

        wsb = {}
        for nm, sh in wnames:
            if sh[0] <= P:
                wsb[nm] = nc.alloc_sbuf_tensor(f"w_{nm}", list(sh), BF16).ap()
                nc.sync.dma_start(wsb[nm], W[nm][:, :])
            else:  # k-tiled along the first (contraction) dim
                tiles = []
                for i, (fo, fk) in enumerate(ktiles(sh[0])):
                    t = nc.alloc_sbuf_tensor(f"w_{nm}{i}", [fk, sh[1]],
                                             BF16).ap()
                    nc.sync.dma_start(t, W[nm][fo:fo + fk, :])
                    tiles.append(t)
                wsb[nm] = tiles
        # att2f / ta2c broadcast to all partitions
        att2bc = nc.alloc_sbuf_tensor("att2bc", [P, H * TOPO], BF16).ap()
        nc.gpsimd.partition_broadcast(att2bc, wsb["att2f"][0:1, :])
        ta2cbc = nc.alloc_sbuf_tensor("ta2cbc", [P, H], BF16).ap()
        nc.gpsimd.partition_broadcast(ta2cbc, wsb["ta2c"][0:1, :])
        ident_bf = nc.alloc_sbuf_tensor("ident_bf", [P, P], BF16).ap()
        nc.vector.tensor_copy(ident_bf, ident)

        # ================= phase A: topo MLP + L1 prep =================
        with tc.tile_pool(name="ppA", bufs=1, space="PSUM") as ppA, \
             tc.tile_pool(name="ppA2", bufs=2, space="PSUM") as ppA2, \
             tc.tile_pool(name="cpA", bufs=3) as cpA, \
             tc.tile_pool(name="spA", bufs=2) as spA:
            # --- topo extractor MLP (feat-major: out rows = hid/topo) ---
            NG = 512
            for go in range(0, NPC, NG):
                gs = min(NG, NPC - go)
                ph = ppA.tile([P, NG], F32, tag="ph", name="ph", space="PSUM")
                for i, (fo, fk) in enumerate(fkt):
                    nc.tensor.matmul(ph[:, :gs], lhsT=wsb["tw1"][i][:fk, :],
                                     rhs=xT_sb[i][:fk, go:go + gs],
                                     start=i == 0, stop=False,
                                     skip_group_check=True)
                nc.tensor.matmul(ph[:, :gs], lhsT=wsb["tb1"][:, :],
                                 rhs=ones_row[:, go:go + gs], start=False,
                                 stop=True, skip_group_check=True)
                t_hid = spA.tile([P, NG], BF16, tag="t_hid", name="t_hid")
                nc.scalar.activation(t_hid[:, :gs], ph[:, :gs], AF.Relu)
                pt = ppA.tile([TOPO, NG], F32, tag="pt", name="pt", space="PSUM")
                nc.tensor.matmul(pt[:, :gs], lhsT=wsb["tw2"][:, :],
                                 rhs=t_hid[:, :gs], start=True, stop=False,
                                 skip_group_check=True)
                nc.tensor.matmul(pt[:, :gs], lhsT=wsb["tb2"][:, :],
                                 rhs=ones_row[:, go:go + gs], start=False,
                                 stop=True, skip_group_check=True)
                nc.vector.tensor_copy(topoT0[:, go:go + gs], pt[:, :gs])

            # --- L1 prep per block ---
            for bi, (bo, bs) in enumerate(blocks):
                pm = ppA2.tile([P, HC], F32, tag="pm", name="pm", space="PSUM")
                pa = ppA.tile([P, 2 * H], F32, tag="pa", name="pa", space="PSUM")
                for i, (fo, fk) in enumerate(fkt):
                    nc.tensor.matmul(pm[:bs, :], lhsT=xT_sb[i][:fk, bo:bo + bs],
                                     rhs=wsb["wl1"][i][:fk, :],
                                     start=i == 0, stop=False,
                                     skip_group_check=True)
                    nc.tensor.matmul(pa[:bs, :], lhsT=xT_sb[i][:fk, bo:bo + bs],
                                     rhs=wsb["A1"][i][:fk, :],
                                     start=i == 0, stop=False,
                                     skip_group_check=True)
                nc.tensor.matmul(pm[:bs, :], lhsT=ones_row[:, bo:bo + bs],
                                 rhs=wsb["bl1"][:, :], start=False, stop=True,
                                 skip_group_check=True)
                nc.tensor.matmul(pa[:bs, :], lhsT=ones_row[:, bo:bo + bs],
                                 rhs=wsb["bA1"][:, :], start=False, stop=True,
                                 skip_group_check=True)
                pta = ppA.tile([P, H], F32, tag="pta", name="pta", space="PSUM")
                nc.tensor.matmul(pta[:bs, :], lhsT=topoT0[:, bo:bo + bs],
                                 rhs=wsb["att2T1"][:, :], start=True,
                                 stop=True, skip_group_check=True)
                ptt = ppA.tile([P, TOPO], BF16, tag="ptt", name="ptt",
                               space="PSUM")
                nc.tensor.transpose(ptt[:bs, :TOPO],
                                    topoT0[:, bo:bo + bs],
                                    ident_bf[:TOPO, :TOPO])
                ext = cpA.tile([P, ROW], F8, tag="ext", name="ext")
                extb = ext[:, :].bitcast(BF16)
                nc.scalar.copy(ext[:bs, 0:HC], pm[:bs, :])
                nc.scalar.copy(extb[:bs, cfg.C_TOPO:cfg.C_TOPO + TOPO],
                               ptt[:bs, :TOPO])
                nc.vector.memset(extb[:bs, cfg.C_ONE:cfg.C_ONE + 1], 1.0)
                nc.scalar.copy(extb[:bs, cfg.C_AL:cfg.C_AL + H], pa[:bs, 0:H])
                nc.scalar.copy(extb[:bs, cfg.C_TA:cfg.C_TA + H], pta[:bs, :])
                nc.sync.dma_start(ext_sl[0][bo:bo + bs, :], ext[:bs, :])
                if debug and bi == 0:
                    da = cpA.tile([P, 24], F32, tag="dbga", name="dbga")
                    nc.vector.tensor_copy(da[:, :],
                                          extb[:, cfg.C_TOPO:cfg.C_TOPO + 24])
                    nc.sync.dma_start(dbg["dbg_aux"][:, :], da[:, :])
                # dst-side rows: [ar | ta]
                nc.vector.tensor_copy(datt1[:bs, bi * 2 * H:bi * 2 * H + H],
                                      pa[:bs, H:2 * H])
                nc.vector.tensor_copy(
                    datt1[:bs, bi * 2 * H + H:(bi + 1) * 2 * H], pta[:bs, :])
            nc.gpsimd.collective_compute(
                "AllGather", OP.bypass, replica_groups=groups,
                ins=[ext_sl[0][:, :]], outs=[ext_fl[0][:, :]])

        # ================= edge phase (shared emitter) =================
        TMAX = max(schedule)

        def emit_edge(L, gp, sp, pp, pp2):
            AUXW = 2 * H if L == 1 else H      # lg width per tile
            AUXO = cfg.C_AL if L == 1 else cfg.C_TA2
            for bi, (bo, bs) in enumerate(blocks):
                Tb = schedule[bi]
                base = int(offs[bi])
                TW = Tb * P
                # ---- gathers ----
                G = gp.tile([P, TMAX * ROW], F8, tag="G", name="G")
                for go in range(0, Tb, GT_MAX):
                    gn = min(GT_MAX, Tb - go)
                    c0 = (base + go) * 8
                    nc.gpsimd.dma_gather(
                        G[:, go * ROW:(go + gn) * ROW].rearrange(
                            "p (t e) -> p t e", e=ROW),
                        ext_fl[L - 1][:, :], src_sb[:, c0:c0 + 8 * gn],
                        num_idxs=P * gn, num_idxs_reg=P * gn, elem_size=ROW,
                        queue_num=0)
                # ---- St / StT (host-built one-hot tables) ----
                stt = sp.tile([P, TMAX * 2 * P], BF16, tag="stt", name="stt")
                nc.sync.dma_start(stt[:, 0:Tb * 2 * P],
                                  stt_i[:, base * 2 * P:(base + Tb) * 2 * P])

                def St(t):
                    return stt[:, t * 2 * P:t * 2 * P + P]

                def StT(t):
                    return stt[:, t * 2 * P + P:(t + 1) * 2 * P]
                # ---- dst-logit lookup ----
                pD = pp.tile([P, TMAX * AUXW], F32, tag="pD", name="pD",
                             space="PSUM")
                dsl = (datt1[:, bi * 2 * H:(bi + 1) * 2 * H] if L == 1
                       else datt2[:, bi * H:(bi + 1) * H])
                for t in range(Tb):
                    nc.tensor.matmul(pD[:, t * AUXW:(t + 1) * AUXW],
                                     lhsT=StT(t), rhs=dsl, start=True,
                                     stop=True, skip_group_check=True)
                # ---- batched logits ----
                Gb = G[:, 0:Tb * ROW].bitcast(BF16).rearrange(
                    "p (t e) -> p t e", e=cfg.RB)
                lg = sp.tile([P, TMAX * AUXW], F32, tag="lg", name="lg")
                nc.vector.tensor_tensor(
                    lg[:, 0:Tb * AUXW].rearrange("p (t c) -> p t c", c=AUXW),
                    Gb[:, :, AUXO:AUXO + AUXW],
                    pD[:, 0:Tb * AUXW].rearrange("p (t c) -> p t c", c=AUXW),
                    OP.add)
                lr = sp.tile([P, TMAX * AUXW], F32, tag="lr", name="lr")
                nc.vector.scalar_tensor_tensor(
                    lr[:, 0:Tb * AUXW], lg[:, 0:Tb * AUXW], cfg.NEG,
                    lg[:, 0:Tb * AUXW], OP.mult, OP.max)
                et = sp.tile([P, TMAX * AUXW], BF16, tag="et", name="et")
                nc.scalar.activation(et[:, 0:Tb * AUXW], lr[:, 0:Tb * AUXW],
                                     AF.Exp)
                etv = et[:, 0:Tb * AUXW].rearrange("p (t c) -> p t c", c=AUXW)
                # ---- weighted messages (split DVE / gpsimd) ----
                Gp = gp.tile([P, TMAX * HC], BF16, tag="Gp", name="Gp")
                e2off = H if L == 1 else 0
                Gf = G[:, 0:Tb * ROW].rearrange("p (t e) -> p t e", e=ROW)
                ks = (Tb * GP_POOL_FRAC + 9) // 10  # first ks tiles on Pool

                def gp_op(eng, t0, t1):
                    if t1 <= t0:
                        return
                    eng.tensor_tensor(
                        Gp[:, t0 * HC:t1 * HC].rearrange(
                            "p (t h c) -> p t h c", h=H, c=HID),
                        Gf[:, t0:t1, 0:HC].rearrange(
                            "p t (h c) -> p t h c", c=HID),
                        etv[:, t0:t1, e2off:e2off + H].unsqueeze(
                            3).to_broadcast((P, t1 - t0, H, HID)),
                        OP.mult)
                gp_op(nc.gpsimd, 0, ks)
                gp_op(nc.vector, ks, Tb)
                if L == 1:
                    SMW = 16 * H + H
                    SMe = sp.tile([P, TMAX * SMW], BF16, tag="SMe", name="SMe")
                    SMv = SMe[:, 0:Tb * SMW].rearrange("p (t c) -> p t c",
                                                       c=SMW)
                    nc.vector.tensor_tensor(
                        SMv[:, :, 0:16 * H].rearrange(
                            "p t (h j) -> p t h j", j=16),
                        Gb[:, :, cfg.C_TOPO:cfg.C_TOPO + 16].unsqueeze(
                            2).to_broadcast((P, Tb, H, 16)),
                        etv[:, :, 0:H].unsqueeze(3).to_broadcast(
                            (P, Tb, H, 16)),
                        OP.mult)
                    nc.scalar.copy(SMv[:, :, 16 * H:SMW],
                                   etv[:, :, H:2 * H])
                else:
                    SMW = H
                    SMe = et
                # ---- aggregation matmuls ----
                pf = pp2.tile([P, HC], F32, tag="pf", name="pf", space="PSUM")
                psm = pp.tile([P, SMW], F32, tag="psm", name="psm",
                              space="PSUM")
                for t in range(Tb):
                    st0, sp1 = t == 0, t == Tb - 1
                    nc.tensor.matmul(pf[:, :], lhsT=St(t),
                                     rhs=Gp[:, t * HC:(t + 1) * HC],
                                     start=st0, stop=sp1,
                                     skip_group_check=True)
                    nc.tensor.matmul(psm[:, :], lhsT=St(t),
                                     rhs=SMe[:, t * SMW:(t + 1) * SMW],
                                     start=st0, stop=sp1,
                                     skip_group_check=True)
                if debug and bi == 0:
                    dt = sp.tile([P, 68], F32, tag="dbgp", name="dbgp")
                    nc.vector.tensor_copy(dt[:, 0:SMW], psm[:, :])
                    nc.sync.dma_start(
                        dbg["dbg_psm1" if L == 1 else "dbg_psm2"][:, 0:SMW],
                        dt[:, 0:SMW])
                    dp = sp.tile([P, 2 * H], F32, tag="dbgd", name="dbgd")
                    nc.vector.tensor_copy(dp[:, 0:AUXW], pD[:, 0:AUXW])
                    if L == 1:
                        nc.sync.dma_start(dbg["dbg_pd1"][:, 0:AUXW],
                                          dp[:, 0:AUXW])
                # ---- drain ----
                if L == 1:
                    drain1(bi, bo, bs, pf, psm, sp, pp, pp2)
                else:
                    drain2(bi, bo, bs, pf, psm, sp, pp)

        # ---- L1 drain + fused L2 prep ----
        def drain1(bi, bo, bs, pf, psm, sp, pp, pp2):
            # rec2 = 1/sum(e2), rec1' = 1/(H*sum(e1))
            den = sp.tile([P, 2 * H], F32, tag="den", name="den")
            nc.vector.tensor_scalar(
                den[:, 0:H].unsqueeze(2),
                psm[:, 0:16 * H].rearrange("p (h j) -> p h j", j=16)[
                    :, :, 15:16],
                float(H), eps_col[:, 0:1], OP.mult, OP.max)
            nc.vector.tensor_tensor(den[:, H:2 * H], psm[:, 16 * H:16 * H + H],
                                    eps_col[:, 0:1].to_broadcast((P, H)),
                                    OP.max)
            rec = sp.tile([P, 2 * H], F32, tag="rec", name="rec")
            nc.vector.reciprocal(rec[:, :], den[:, :])
            # h1 = agg_feat * rec2 (node-major, bf16; per-head scale on Act)
            h1 = sp.tile([P, HC], BF16, tag="h1", name="h1")
            for h in range(H):
                nc.scalar.activation(h1[:, h * HID:(h + 1) * HID],
                                     pf[:, h * HID:(h + 1) * HID], AF.Copy,
                                     scale=rec[:, H + h:H + h + 1])
            # topo1_raw = sum_h agg_topo_h * rec1'   [d, 15]
            tp = sp.tile([P, TOPO * H], F32, tag="tp", name="tp")
            nc.vector.tensor_tensor(
                tp[:, :].rearrange("p (j h) -> p j h", h=H),
                psm[:, 0:16 * H].rearrange("p (h j) -> p h j", j=16)[
                    :, :, 0:TOPO].transpose([0, 2, 1]),
                rec[:, 0:H].unsqueeze(1).to_broadcast((P, TOPO, H)),
                OP.mult)
            t1 = sp.tile([P, TOPO], F32, tag="t1", name="t1")
            nc.vector.tensor_reduce(
                t1[:, :], tp[:, :].rearrange("p (j h) -> p j h", h=H),
                mybir.AxisListType.X, OP.add)
            # ta2 = topo1_raw @ att2T2 (per-node, via DVE reduce)
            tq = sp.tile([P, H * TOPO], F32, tag="tq", name="tq")
            nc.vector.tensor_tensor(
                tq[:, :].rearrange("p (h j) -> p h j", j=TOPO),
                t1[:, :].unsqueeze(1).to_broadcast((P, H, TOPO)),
                att2bc[:, :].rearrange("p (h j) -> p h j", j=TOPO),
                OP.mult)
            ta2 = sp.tile([P, H], F32, tag="ta2", name="ta2")
            nc.vector.tensor_reduce(
                ta2[:, :], tq[:, :].rearrange("p (h j) -> p h j", j=TOPO),
                mybir.AxisListType.X, OP.add)
            # dst rows for L2: ta2 + 2*(b2@att2)
            nc.vector.tensor_tensor(datt2[:bs, bi * H:(bi + 1) * H],
                                    ta2[:bs, :],
                                    ta2cbc[:bs, :], OP.add)
            if debug and bi == 0:
                dh = sp.tile([P, HC], F32, tag="dbgh", name="dbgh")
                nc.vector.tensor_copy(dh[:, :], h1[:, :])
                nc.sync.dma_start(dbg["dbg_h1"][:, :], dh[:, :])
                dtt = sp.tile([P, TOPO + H], F32, tag="dbgt", name="dbgt")
                nc.vector.tensor_copy(dtt[:, 0:TOPO], t1[:, :])
                nc.vector.tensor_copy(dtt[:, TOPO:TOPO + H], ta2[:, :])
                nc.sync.dma_start(dbg["dbg_tt"][:, :], dtt[:, :])
            # transpose h1 -> hfmT tiles
            for ci, (co, ck) in enumerate(ckt):
                ptr = pp.tile([P, P], BF16, tag="ptr", name="ptr",
                              space="PSUM")
                nc.tensor.transpose(ptr[:ck, :bs], h1[:bs, co:co + ck],
                                    ident_bf[:bs, :bs])
                nc.scalar.copy(hfmT[ci][:ck, bo:bo + bs], ptr[:ck, :bs])
            # ---- fused L2 prep for this block ----
            pm2 = pp2.tile([P, HC], F32, tag="pm2", name="pm2", space="PSUM")
            for ci, (co, ck) in enumerate(ckt):
                nc.tensor.matmul(pm2[:bs, :], lhsT=hfmT[ci][:ck, bo:bo + bs],
                                 rhs=wsb["wl2"][ci][:ck, :],
                                 start=ci == 0, stop=False,
                                 skip_group_check=True)
            nc.tensor.matmul(pm2[:bs, :], lhsT=ones_row[:, bo:bo + bs],
                             rhs=wsb["bl2"][:, :], start=False, stop=True,
                             skip_group_check=True)
            ext = sp.tile([P, ROW], F8, tag="ext2", name="ext2")
            nc.scalar.copy(ext[:bs, 0:HC], pm2[:bs, :])
            nc.scalar.copy(ext[:, :].bitcast(BF16)[
                :bs, cfg.C_TA2:cfg.C_TA2 + H], ta2[:bs, :])
            nc.sync.dma_start(ext_sl[1][bo:bo + bs, :], ext[:bs, :])

        # ---- L2 drain: normalize + pooled partial ----
        def drain2(bi, bo, bs, pf, psm, sp, pp):
            den = sp.tile([P, H], F32, tag="den2", name="den2")
            nc.vector.tensor_tensor(den[:, :], psm[:, 0:H],
                                    eps_col[:, 0:1].to_broadcast((P, H)),
                                    OP.max)
            rec = sp.tile([P, H], F32, tag="rec2", name="rec2")
            nc.vector.reciprocal(rec[:, :], den[:, :])
            h2 = sp.tile([P, HC], BF16, tag="h2", name="h2")
            for h in range(H):
                nc.scalar.activation(h2[:, h * HID:(h + 1) * HID],
                                     pf[:, h * HID:(h + 1) * HID], AF.Copy,
                                     scale=rec[:, h:h + 1])
            if debug and bi == 0:
                dh = sp.tile([P, HC], F32, tag="dbgh2", name="dbgh2")
                nc.vector.tensor_copy(dh[:, :], h2[:, :])
                nc.sync.dma_start(dbg["dbg_h2"][:, :], dh[:, :])
            nc.tensor.matmul(pr.pool_ps[:, :], lhsT=ones_col[:bs, 0:1],
                             rhs=h2[:bs, :], start=bi == 0,
                             stop=bi == len(blocks) - 1,
                             skip_group_check=True)

        # ================= phase B/C: L1 edges (+L2 prep) =================
        # PSUM banks: pf 2 + pm2 2 + pD/psm/ptr 1 each = 7 of 8
        with tc.tile_pool(name="gpB", bufs=2) as gpB, \
             tc.tile_pool(name="spB", bufs=2) as spB, \
             tc.tile_pool(name="ppB", bufs=1, space="PSUM") as ppB, \
             tc.tile_pool(name="ppB2", bufs=2, space="PSUM") as ppB2:
            emit_edge(1, gpB, spB, ppB, ppB2)
            nc.gpsimd.collective_compute(
                "AllGather", OP.bypass, replica_groups=groups,
                ins=[ext_sl[1][:, :]], outs=[ext_fl[1][:, :]])

        # ================= phase D: L2 edges =================
        with tc.tile_pool(name="gpD", bufs=2) as gpD, \
             tc.tile_pool(name="spD", bufs=2) as spD, \
             tc.tile_pool(name="ppD", bufs=1, space="PSUM") as ppD, \
             tc.tile_pool(name="ppD2", bufs=2, space="PSUM") as ppD2, \
             tc.tile_pool(name="plD", bufs=1, space="PSUM") as plD:
            pr.pool_ps = plD.tile([1, HC], F32, tag="pool", name="pool",
                                  space="PSUM", bufs=1)
            emit_edge(2, gpD, spD, ppD, ppD2)

            # ---- pool + heads ----
            pooled = spD.tile([1, HC], F32, tag="pooled", name="pooled")
            nc.vector.tensor_copy(pooled[:, :], pr.pool_ps[:, :])
            nc.sync.dma_start(pool_in[:, :], pooled[:, :])
            nc.gpsimd.collective_compute(
                "AllReduce", OP.add, replica_groups=groups,
                ins=[pool_in[:, :]], outs=[pool_out[:, :]])
            # load back column-major: pmean_cols[c, h] = pool_out[h*HID+c]
            pooled2 = spD.tile([P, H], F32, tag="pooled2", name="pooled2")
            with nc.allow_non_contiguous_dma("pool row -> col-major reload"):
                nc.sync.dma_start(
                    pooled2[:, :],
                    pool_out[:, :].rearrange("o (h c) -> (o c) h", c=HID))
            if debug:
                nc.sync.dma_start(dbg["dbg_pool"][:, :], pooled2[:, :])
            pmean = spD.tile([P, H], BF16, tag="pmean", name="pmean")
            nc.vector.tensor_scalar(pmean[:, :], pooled2[:, :], 1.0 / N,
                                    None, OP.mult)
            for nm, out_t in (("v", val_o), ("a", aro_o)):
                pm = ppD.tile([P, 1], F32, tag="mlp", name="mlp", space="PSUM")
                for ki in range(H):
                    nc.tensor.matmul(pm[:, :], lhsT=wsb[f"{nm}w1"][ki][:, :],
                                     rhs=pmean[:, ki:ki + 1], start=ki == 0,
                                     stop=False, skip_group_check=True)
                nc.tensor.matmul(pm[:, :], lhsT=wsb[f"{nm}b1"][:, :],
                                 rhs=ones_col[0:1, :], start=False, stop=True,
                                 skip_group_check=True)
                hv = spD.tile([P, 1], BF16, tag=f"{nm}hv", name=f"{nm}hv")
                nc.scalar.activation(hv[:, :], pm[:, :], AF.Relu)
                po = ppD.tile([1, 1], F32, tag="mlpo", name="mlpo",
                              space="PSUM")
                nc.tensor.matmul(po[:, :], lhsT=hv[:, :],
                                 rhs=wsb[f"{nm}w2"][:, :], start=True,
                                 stop=False, skip_group_check=True)
                nc.tensor.matmul(po[:, :], lhsT=wsb[f"{nm}b2"][:, :],
                                 rhs=ones_col[0:1, :], start=False, stop=True,
                                 skip_group_check=True)
                ov = spD.tile([1, 1], F32, tag=f"{nm}ov", name=f"{nm}ov")
                nc.vector.tensor_copy(ov[:, :], po[:, :])
                nc.sync.dma_start(out_t[:, :], ov[:, :])

    nc.compile()
    es.close()
    return pr


# --------------------------------------------------------------------------
# entry point
# --------------------------------------------------------------------------

_CACHE = {}


def make_in_maps(inputs, cfg, src_w, stt):
    x = np.asarray(inputs["x"], dtype=np.float32)
    shared = host_weights(inputs, cfg)
    in_maps = []
    for c in range(cfg.CORES):
        m = dict(shared)
        m["xT_slice"] = np.ascontiguousarray(
            x[c * cfg.NPC:(c + 1) * cfg.NPC].T.astype(BF))
        m["src_idx"] = np.ascontiguousarray(src_w[c])
        m["stt_tab"] = stt[c]
        in_maps.append(m)
    return in_maps


def run(inputs, cfg=CFG, trace=False):
    schedule, src_w, stt = host_prep(inputs["edge_index"], cfg)
    key = (cfg.N, cfg.E, tuple(schedule))
    if key not in _CACHE:
        _CACHE[key] = build_program(cfg, schedule)
    pr = _CACHE[key]
    in_maps = make_in_maps(inputs, cfg, src_w, stt)
    res = run_bass_kernel_spmd(pr.nc, in_maps, list(range(cfg.CORES)),
                               trace=trace)
    out = res.results[0]
    return (np.asarray(out["valence"], np.float32),
            np.asarray(out["arousal"], np.float32)), res


def kernel(**inputs):
    (val, aro), _ = run(inputs)
    return (val, aro)


# revision 57
# speedup vs baseline: 1.5878x; 1.0436x over previous
"""GCATopo (2-layer GTAT GNN) Trainium2 kernel, 8-way SPMD — v2.

Strategy (v2 redesign vs v1):
 - Node-major aggregation: per 128-edge tile ONE 512-wide matmul
   (lhsT=St one-hot, rhs=et2-weighted gathered features) accumulates
   [dst, 512] in a single PSUM bank; softmax denominators aggregate in a
   second small matmul. Normalization becomes per-partition scaling.
 - Per-edge dst logits come from a lookup matmul (lhsT=StT, rhs=local
   per-block dst-attn rows) instead of a 256B-per-edge DMA gather.
 - All per-edge elementwise work (logits, leaky-relu, exp, message
   weighting) is batched across a block's ~14 tiles with strided 3D/4D
   APs — a handful of DVE/Act instructions per block instead of ~15 per
   tile.
 - L2's topo output is discarded by the model, so L2 ships only
   [feat 512 | ta 4] and skips the SM stream entirely.
 - All matmul operands bf16 (4x PE rate vs f32); weights are host-folded
   (wl@attB etc.) and host-transposed; x arrives pre-transposed bf16.
 - Biases are folded forward into the next layer's constant rows, so
   drains are pure scaling.
 - L2 prep is fused into the L1 edge-phase block loop (PE prep matmuls
   overlap DVE/DMA edge work).
"""

from contextlib import ExitStack

import ml_dtypes
import numpy as np

import concourse.bacc as bacc
import concourse.tile as tile
from concourse import mybir
from concourse.masks import make_identity
from concourse.bass_utils import run_bass_kernel_spmd

F32 = mybir.dt.float32
BF16 = mybir.dt.bfloat16
F8 = mybir.dt.float8e4
I16 = mybir.dt.int16
AF = mybir.ActivationFunctionType
OP = mybir.AluOpType

P = 128
BF = ml_dtypes.bfloat16


class Cfg:
    def __init__(self, N=20000, E=240000, FIN=576, HID=128, TOPO=15, H=4,
                 CORES=8, NEG=0.2):
        self.N, self.E, self.FIN, self.HID, self.TOPO, self.H = N, E, FIN, HID, TOPO, H
        self.CORES, self.NEG = CORES, NEG
        self.HC = H * HID                      # 512
        self.ROW = 768                         # gathered row: fp8 feat + pad
        self.RB = self.ROW // 2                # bf16 view width (384)
        self.NPC = N // CORES                  # nodes per core
        self.NBLK = (self.NPC + P - 1) // P    # dst blocks per core
        # aux slots within the BF16 VIEW of the row (bf16 element offsets;
        # feat occupies bf16-view [0:256))
        self.C_TOPO = 256                      # 256..270: topo (L1)
        self.C_ONE = 256 + TOPO                # 271: constant 1.0 (L1)
        self.C_AL = 272                        # 272..275: al (L1)
        self.C_TA = 276                        # 276..279: ta (L1)
        self.C_TA2 = 256                       # 256..259: ta (L2)


CFG = Cfg()
GT_MAX = 8  # max tiles (=128 idxs each) per gather call
GP_POOL_PCT = 25  # percent of Gp tiles offloaded DVE -> gpsimd


def cdiv(a, b):
    return (a + b - 1) // b


def ktiles(F):
    return [(o, min(P, F - o)) for o in range(0, F, P)]


# --------------------------------------------------------------------------
# host-side graph preprocessing (pure indexing on edge_index)
# --------------------------------------------------------------------------

def host_prep(edge_index, cfg):
    N, CORES, NPC, NBLK = cfg.N, cfg.CORES, cfg.NPC, cfg.NBLK
    src = np.asarray(edge_index[0], dtype=np.int64)
    dst = np.asarray(edge_index[1], dtype=np.int64)
    loops = np.arange(N, dtype=np.int64)
    src = np.concatenate([src, loops])
    dst = np.concatenate([dst, loops])
    order = np.argsort(dst, kind="stable")
    s, d = src[order], dst[order]

    core_of = d // NPC
    blk_of = (d % NPC) // P
    counts = np.zeros((CORES, NBLK), dtype=np.int64)
    for c in range(CORES):
        m = core_of == c
        bb = blk_of[m]
        for b in range(NBLK):
            counts[c, b] = int((bb == b).sum())
    schedule = [max(1, cdiv(int(counts[:, b].max()), P)) for b in range(NBLK)]
    offs = np.concatenate([[0], np.cumsum(schedule)]).astype(np.int64)
    ttot = int(offs[-1])

    srcidx = np.zeros((CORES, ttot * P), dtype=np.int16)
    dstloc = np.full((CORES, ttot * P), -1.0, dtype=np.float32)
    for c in range(CORES):
        m = core_of == c
        sc, dc, bc = s[m], d[m], blk_of[m]
        for b in range(NBLK):
            mb = bc == b
            n = int(mb.sum())
            base = int(offs[b]) * P
            srcidx[c, base:base + n] = sc[mb].astype(np.int16)
            dstloc[c, base:base + n] = (dc[mb] - (c * NPC + b * P)).astype(np.float32)

    # wrap for dma_gather: index i lives at [i % 16, i // 16]; the 16-row
    # block is replicated 8x along partitions (one stripe per gpsimd core)
    src_w = [np.tile(srcidx[c].reshape(-1, 16).T, (8, 1)).copy()
             for c in range(CORES)]
    # host-built one-hot selection tables, per tile [St | StT] (bf16 0/1):
    #   St[e, d] = (dstloc[e] == d), StT = St^T
    rng = np.arange(P, dtype=np.float32)
    stt = []
    for c in range(CORES):
        dl = dstloc[c].reshape(ttot, P)
        St = (dl[:, :, None] == rng[None, None, :])          # [t, e, d]
        tab = np.concatenate([St, St.transpose(0, 2, 1)], 2)  # [t, p, 256]
        stt.append(np.ascontiguousarray(
            tab.transpose(1, 0, 2).reshape(P, ttot * 2 * P).astype(BF)))
    return schedule, src_w, stt


def host_weights(inputs, cfg):
    """All small-weight folding in f32 numpy, shipped as bf16."""
    H, C, TOPO, HC = cfg.H, cfg.HID, cfg.TOPO, cfg.HC
    f = lambda k: np.asarray(inputs[k], np.float32)

    def attB(att):  # [1,H,C] -> block-diag [H*C, H]
        out = np.zeros((H * C, H), np.float32)
        a = np.asarray(att, np.float32).reshape(H, C)
        for h in range(H):
            out[h * C:(h + 1) * C, h] = a[h]
        return out

    w = {}
    # topo extractor
    w["tw1"] = f("te_w1")                      # [576,128]
    w["tb1"] = f("te_b1").reshape(1, -1)
    w["tw2"] = f("te_w2")                      # [128,15]
    w["tb2"] = f("te_b2").reshape(1, -1)
    # layer 1
    aB1 = attB(inputs["l1_att"])
    w["wl1"] = f("l1_wl")                      # [576,512]
    w["bl1"] = f("l1_bl").reshape(1, -1)
    w["A1"] = np.concatenate([f("l1_wl") @ aB1, f("l1_wr") @ aB1], 1)  # [576,8]
    w["bA1"] = np.concatenate([f("l1_bl") @ aB1, f("l1_br") @ aB1]).reshape(1, -1)
    w["att2T1"] = f("l1_att2").reshape(H, TOPO).T      # [15,4]
    # layer 2 (input h1 = agg1_norm, l1_bias folded here)
    b1 = f("l1_bias")
    w["wl2"] = f("l2_wl")                      # [512,512]
    w["bl2"] = (b1 @ f("l2_wl") + f("l2_bl")).reshape(1, -1)
    w["att2T2"] = f("l2_att2").reshape(H, TOPO).T      # [15,4]
    # topo1 input to L2 = topo1_raw + l1_bias2; edge logit gets the const
    # twice (src+dst) -> fold 2*(b2@att2) into the dst-side rows only
    w["ta2c"] = (2.0 * (f("l1_bias2") @ w["att2T2"])).reshape(1, -1)   # [1,4]
    # heads (l2_bias folded into first-layer bias)
    b2f = f("l2_bias")
    for nm in ("v", "a"):
        w[f"{nm}w1"] = f(f"{nm}_w1")           # [512,128]
        w[f"{nm}b1"] = (f(f"{nm}_b1") + b2f @ f(f"{nm}_w1")).reshape(1, -1)
        w[f"{nm}w2"] = f(f"{nm}_w2")           # [128,1]
        w[f"{nm}b2"] = f(f"{nm}_b2").reshape(1, 1)
    # att2T2 flattened (h,j) row for the drain's ta2 reduce + const
    w["att2f"] = w["att2T2"].T.reshape(1, -1)  # [1,60] (h-major)
    return {k: v.astype(BF) for k, v in w.items()}


# --------------------------------------------------------------------------
# program builder
# --------------------------------------------------------------------------

class Prog:
    pass


def build_program(cfg, schedule, debug=False):
    es = ExitStack()
    nc = bacc.Bacc("TRN2", target_bir_lowering=False, debug=False,
                   num_devices=cfg.CORES)
    pr = Prog()
    pr.nc = nc
    N, FIN, HID, TOPO, H, HC, ROW, NPC, NBLK = (
        cfg.N, cfg.FIN, cfg.HID, cfg.TOPO, cfg.H, cfg.HC, cfg.ROW, cfg.NPC,
        cfg.NBLK)
    TTOT = sum(schedule)
    W16 = TTOT * P // 16
    groups = [list(range(cfg.CORES))]
    blocks = ktiles(NPC)
    fkt = ktiles(FIN)
    ckt = ktiles(HC)
    offs = np.concatenate([[0], np.cumsum(schedule)]).astype(int)

    def din(name, shape, dtype=BF16):
        return nc.dram_tensor(name, list(shape), dtype, kind="ExternalInput")

    # ---- external inputs ----
    xT = din("xT_slice", (FIN, NPC))
    wnames = [("tw1", (FIN, HID)), ("tb1", (1, HID)), ("tw2", (HID, TOPO)),
              ("tb2", (1, TOPO)), ("wl1", (FIN, HC)), ("bl1", (1, HC)),
              ("A1", (FIN, 2 * H)), ("bA1", (1, 2 * H)), ("att2T1", (TOPO, H)),
              ("wl2", (HC, HC)), ("bl2", (1, HC)), ("att2T2", (TOPO, H)),
              ("ta2c", (1, H)), ("att2f", (1, H * TOPO)),
              ("vw1", (HC, HID)), ("vb1", (1, HID)), ("vw2", (HID, 1)),
              ("vb2", (1, 1)),
              ("aw1", (HC, HID)), ("ab1", (1, HID)), ("aw2", (HID, 1)),
              ("ab2", (1, 1))]
    W = {nm: din(nm, sh) for nm, sh in wnames}
    src_i = din("src_idx", (P, W16), I16)
    stt_i = din("stt_tab", (P, TTOT * 2 * P))

    # ---- outputs ----
    val_o = nc.dram_tensor("valence", [1, 1], F32, kind="ExternalOutput")
    aro_o = nc.dram_tensor("arousal", [1, 1], F32, kind="ExternalOutput")
    dbg = {}
    if debug:
        for nm, sh in [("dbg_h1", (P, HC)), ("dbg_tt", (P, TOPO + H)),
                       ("dbg_psm1", (P, 68)), ("dbg_h2", (P, HC)),
                       ("dbg_pool", (P, H)), ("dbg_aux", (P, 24)),
                       ("dbg_psm2", (P, H)), ("dbg_pd1", (P, 2 * H))]:
            dbg[nm] = nc.dram_tensor(nm, list(sh), F32, kind="ExternalOutput")

    # ---- internal DRAM ----
    ext_sl = [nc.dram_tensor(f"ext_slice{L}", [NPC, ROW], F8)
              for L in (1, 2)]
    ext_fl = [nc.dram_tensor(f"ext_full{L}", [N, ROW], F8,
                             addr_space="Shared") for L in (1, 2)]
    pool_in = nc.dram_tensor("pool_in", [1, HC], F32)
    pool_out = nc.dram_tensor("pool_out", [1, HC], F32, addr_space="Shared")

    with tile.TileContext(nc) as tc:
        # ================= static SBUF =================
        ident = nc.alloc_sbuf_tensor("ident", [P, P], F32).ap()
        make_identity(nc, ident)
        ones_row = nc.alloc_sbuf_tensor("ones_row", [1, NPC], BF16).ap()
        nc.gpsimd.memset(ones_row, 1.0)
        ones_col = nc.alloc_sbuf_tensor("ones_col", [P, 1], BF16).ap()
        nc.gpsimd.memset(ones_col, 1.0)
        eps_col = nc.alloc_sbuf_tensor("eps_col", [P, 1], F32).ap()
        nc.gpsimd.memset(eps_col, 1e-30)

        src_sb = nc.alloc_sbuf_tensor("src_sb", [P, W16], I16).ap()
        nc.sync.dma_start(src_sb, src_i[:, :])

        # resident activations / weights
        xT_sb = [nc.alloc_sbuf_tensor(f"xT{i}", [P, NPC], BF16).ap()
                 for i in range(len(fkt))]
        for i, (fo, fk) in enumerate(fkt):
            nc.sync.dma_start(xT_sb[i][:fk, :], xT[fo:fo + fk, :])
        hfmT = [nc.alloc_sbuf_tensor(f"hfmT{i}", [P, NPC], BF16).ap()
                for i in range(len(ckt))]
        topoT0 = nc.alloc_sbuf_tensor("topoT0", [TOPO, NPC], BF16).ap()
        datt1 = nc.alloc_sbuf_tensor("datt1", [P, NBLK * 2 * H], BF16).ap()
        datt2 = nc.alloc_sbuf_tensor("datt2", [P, NBLK * H], BF16).ap()
        nc.vector.memset(datt1, 0.0)   # rows past a partial block stay 0
        nc.vector.memset(datt2, 0.0)

        wsb = {}
        for nm, sh in wnames:
            if sh[0] <= P:
                wsb[nm] = nc.alloc_sbuf_tensor(f"w_{nm}", list(sh), BF16).ap()
                nc.sync.dma_start(wsb[nm], W[nm][:, :])
            else:  # k-tiled along the first (contraction) dim
                tiles = []
                for i, (fo, fk) in enumerate(ktiles(sh[0])):
                    t = nc.alloc_sbuf_tensor(f"w_{nm}{i}", [fk, sh[1]],
                                             BF16).ap()
                    nc.sync.dma_start(t, W[nm][fo:fo + fk, :])
                    tiles.append(t)
                wsb[nm] = tiles
        # att2f / ta2c broadcast to all partitions
        att2bc = nc.alloc_sbuf_tensor("att2bc", [P, H * TOPO], BF16).ap()
        nc.gpsimd.partition_broadcast(att2bc, wsb["att2f"][0:1, :])
        ta2cbc = nc.alloc_sbuf_tensor("ta2cbc", [P, H], BF16).ap()
        nc.gpsimd.partition_broadcast(ta2cbc, wsb["ta2c"][0:1, :])
        ident_bf = nc.alloc_sbuf_tensor("ident_bf", [P, P], BF16).ap()
        nc.vector.tensor_copy(ident_bf, ident)

        # ================= phase A: topo MLP + L1 prep =================
        with tc.tile_pool(name="ppA", bufs=1, space="PSUM") as ppA, \
             tc.tile_pool(name="ppA2", bufs=2, space="PSUM") as ppA2, \
             tc.tile_pool(name="cpA", bufs=3) as cpA, \
             tc.tile_pool(name="spA", bufs=2) as spA:
            # --- topo extractor MLP (feat-major: out rows = hid/topo) ---
            NG = 512
            for go in range(0, NPC, NG):
                gs = min(NG, NPC - go)
                ph = ppA.tile([P, NG], F32, tag="ph", name="ph", space="PSUM")
                for i, (fo, fk) in enumerate(fkt):
                    nc.tensor.matmul(ph[:, :gs], lhsT=wsb["tw1"][i][:fk, :],
                                     rhs=xT_sb[i][:fk, go:go + gs],
                                     start=i == 0, stop=False,
                                     skip_group_check=True)
                nc.tensor.matmul(ph[:, :gs], lhsT=wsb["tb1"][:, :],
                                 rhs=ones_row[:, go:go + gs], start=False,
                                 stop=True, skip_group_check=True)
                t_hid = spA.tile([P, NG], BF16, tag="t_hid", name="t_hid")
                nc.scalar.activation(t_hid[:, :gs], ph[:, :gs], AF.Relu)
                pt = ppA.tile([TOPO, NG], F32, tag="pt", name="pt", space="PSUM")
                nc.tensor.matmul(pt[:, :gs], lhsT=wsb["tw2"][:, :],
                                 rhs=t_hid[:, :gs], start=True, stop=False,
                                 skip_group_check=True)
                nc.tensor.matmul(pt[:, :gs], lhsT=wsb["tb2"][:, :],
                                 rhs=ones_row[:, go:go + gs], start=False,
                                 stop=True, skip_group_check=True)
                nc.vector.tensor_copy(topoT0[:, go:go + gs], pt[:, :gs])

            # --- L1 prep per block ---
            for bi, (bo, bs) in enumerate(blocks):
                pm = ppA2.tile([P, HC], F32, tag="pm", name="pm", space="PSUM")
                pa = ppA.tile([P, 2 * H], F32, tag="pa", name="pa", space="PSUM")
                for i, (fo, fk) in enumerate(fkt):
                    nc.tensor.matmul(pm[:bs, :], lhsT=xT_sb[i][:fk, bo:bo + bs],
                                     rhs=wsb["wl1"][i][:fk, :],
                                     start=i == 0, stop=False,
                                     skip_group_check=True)
                    nc.tensor.matmul(pa[:bs, :], lhsT=xT_sb[i][:fk, bo:bo + bs],
                                     rhs=wsb["A1"][i][:fk, :],
                                     start=i == 0, stop=False,
                                     skip_group_check=True)
                nc.tensor.matmul(pm[:bs, :], lhsT=ones_row[:, bo:bo + bs],
                                 rhs=wsb["bl1"][:, :], start=False, stop=True,
                                 skip_group_check=True)
                nc.tensor.matmul(pa[:bs, :], lhsT=ones_row[:, bo:bo + bs],
                                 rhs=wsb["bA1"][:, :], start=False, stop=True,
                                 skip_group_check=True)
                pta = ppA.tile([P, H], F32, tag="pta", name="pta", space="PSUM")
                nc.tensor.matmul(pta[:bs, :], lhsT=topoT0[:, bo:bo + bs],
                                 rhs=wsb["att2T1"][:, :], start=True,
                                 stop=True, skip_group_check=True)
                ptt = ppA.tile([P, TOPO], BF16, tag="ptt", name="ptt",
                               space="PSUM")
                nc.tensor.transpose(ptt[:bs, :TOPO],
                                    topoT0[:, bo:bo + bs],
                                    ident_bf[:TOPO, :TOPO])
                ext = cpA.tile([P, ROW], F8, tag="ext", name="ext")
                extb = ext[:, :].bitcast(BF16)
                nc.scalar.copy(ext[:bs, 0:HC], pm[:bs, :])
                nc.scalar.copy(extb[:bs, cfg.C_TOPO:cfg.C_TOPO + TOPO],
                               ptt[:bs, :TOPO])
                nc.vector.memset(extb[:bs, cfg.C_ONE:cfg.C_ONE + 1], 1.0)
                nc.scalar.copy(extb[:bs, cfg.C_AL:cfg.C_AL + H], pa[:bs, 0:H])
                nc.scalar.copy(extb[:bs, cfg.C_TA:cfg.C_TA + H], pta[:bs, :])
                nc.sync.dma_start(ext_sl[0][bo:bo + bs, :], ext[:bs, :])
                if debug and bi == 0:
                    da = cpA.tile([P, 24], F32, tag="dbga", name="dbga")
                    nc.vector.tensor_copy(da[:, :],
                                          extb[:, cfg.C_TOPO:cfg.C_TOPO + 24])
                    nc.sync.dma_start(dbg["dbg_aux"][:, :], da[:, :])
                # dst-side rows: [ar | ta]
                nc.vector.tensor_copy(datt1[:bs, bi * 2 * H:bi * 2 * H + H],
                                      pa[:bs, H:2 * H])
                nc.vector.tensor_copy(
                    datt1[:bs, bi * 2 * H + H:(bi + 1) * 2 * H], pta[:bs, :])
            nc.gpsimd.collective_compute(
                "AllGather", OP.bypass, replica_groups=groups,
                ins=[ext_sl[0][:, :]], outs=[ext_fl[0][:, :]])

        # ================= edge phase (shared emitter) =================
        TMAX = max(schedule)

        def emit_edge(L, gp, sp, pp, pp2):
            AUXW = 2 * H if L == 1 else H      # lg width per tile
            AUXO = cfg.C_AL if L == 1 else cfg.C_TA2
            for bi, (bo, bs) in enumerate(blocks):
                Tb = schedule[bi]
                base = int(offs[bi])
                TW = Tb * P
                # ---- gathers ----
                G = gp.tile([P, TMAX * ROW], F8, tag="G", name="G")
                for go in range(0, Tb, GT_MAX):
                    gn = min(GT_MAX, Tb - go)
                    c0 = (base + go) * 8
                    nc.gpsimd.dma_gather(
                        G[:, go * ROW:(go + gn) * ROW].rearrange(
                            "p (t e) -> p t e", e=ROW),
                        ext_fl[L - 1][:, :], src_sb[:, c0:c0 + 8 * gn],
                        num_idxs=P * gn, num_idxs_reg=P * gn, elem_size=ROW,
                        queue_num=0)
                # ---- St / StT (host-built one-hot tables) ----
                stt = sp.tile([P, TMAX * 2 * P], BF16, tag="stt", name="stt")
                nc.sync.dma_start(stt[:, 0:Tb * 2 * P],
                                  stt_i[:, base * 2 * P:(base + Tb) * 2 * P])

                def St(t):
                    return stt[:, t * 2 * P:t * 2 * P + P]

                def StT(t):
                    return stt[:, t * 2 * P + P:(t + 1) * 2 * P]
                # ---- dst-logit lookup ----
                pD = pp.tile([P, TMAX * AUXW], F32, tag="pD", name="pD",
                             space="PSUM")
                dsl = (datt1[:, bi * 2 * H:(bi + 1) * 2 * H] if L == 1
                       else datt2[:, bi * H:(bi + 1) * H])
                for t in range(Tb):
                    nc.tensor.matmul(pD[:, t * AUXW:(t + 1) * AUXW],
                                     lhsT=StT(t), rhs=dsl, start=True,
                                     stop=True, skip_group_check=True)
                # ---- batched logits ----
                Gb = G[:, 0:Tb * ROW].bitcast(BF16).rearrange(
                    "p (t e) -> p t e", e=cfg.RB)
                lg = sp.tile([P, TMAX * AUXW], F32, tag="lg", name="lg")
                nc.vector.tensor_tensor(
                    lg[:, 0:Tb * AUXW].rearrange("p (t c) -> p t c", c=AUXW),
                    Gb[:, :, AUXO:AUXO + AUXW],
                    pD[:, 0:Tb * AUXW].rearrange("p (t c) -> p t c", c=AUXW),
                    OP.add)
                lr = sp.tile([P, TMAX * AUXW], F32, tag="lr", name="lr")
                nc.vector.scalar_tensor_tensor(
                    lr[:, 0:Tb * AUXW], lg[:, 0:Tb * AUXW], cfg.NEG,
                    lg[:, 0:Tb * AUXW], OP.mult, OP.max)
                et = sp.tile([P, TMAX * AUXW], BF16, tag="et", name="et")
                nc.scalar.activation(et[:, 0:Tb * AUXW], lr[:, 0:Tb * AUXW],
                                     AF.Exp)
                etv = et[:, 0:Tb * AUXW].rearrange("p (t c) -> p t c", c=AUXW)
                # ---- weighted messages (split DVE / gpsimd) ----
                Gp = gp.tile([P, TMAX * HC], BF16, tag="Gp", name="Gp")
                e2off = H if L == 1 else 0
                Gf = G[:, 0:Tb * ROW].rearrange("p (t e) -> p t e", e=ROW)
                ks = (Tb * GP_POOL_PCT + 99) // 100  # first ks tiles on Pool

                def gp_op(eng, t0, t1):
                    if t1 <= t0:
                        return
                    eng.tensor_tensor(
                        Gp[:, t0 * HC:t1 * HC].rearrange(
                            "p (t h c) -> p t h c", h=H, c=HID),
                        Gf[:, t0:t1, 0:HC].rearrange(
                            "p t (h c) -> p t h c", c=HID),
                        etv[:, t0:t1, e2off:e2off + H].unsqueeze(
                            3).to_broadcast((P, t1 - t0, H, HID)),
                        OP.mult)
                gp_op(nc.gpsimd, 0, ks)
                gp_op(nc.vector, ks, Tb)
                if L == 1:
                    SMW = 16 * H + H
                    SMe = sp.tile([P, TMAX * SMW], BF16, tag="SMe", name="SMe")
                    SMv = SMe[:, 0:Tb * SMW].rearrange("p (t c) -> p t c",
                                                       c=SMW)
                    nc.vector.tensor_tensor(
                        SMv[:, :, 0:16 * H].rearrange(
                            "p t (h j) -> p t h j", j=16),
                        Gb[:, :, cfg.C_TOPO:cfg.C_TOPO + 16].unsqueeze(
                            2).to_broadcast((P, Tb, H, 16)),
                        etv[:, :, 0:H].unsqueeze(3).to_broadcast(
                            (P, Tb, H, 16)),
                        OP.mult)
                    nc.scalar.copy(SMv[:, :, 16 * H:SMW],
                                   etv[:, :, H:2 * H])
                else:
                    SMW = H
                    SMe = et
                # ---- aggregation matmuls ----
                pf = pp2.tile([P, HC], F32, tag="pf", name="pf", space="PSUM")
                psm = pp2.tile([P, SMW], F32, tag="psm", name="psm",
                               space="PSUM")
                for t in range(Tb):
                    st0, sp1 = t == 0, t == Tb - 1
                    nc.tensor.matmul(pf[:, :], lhsT=St(t),
                                     rhs=Gp[:, t * HC:(t + 1) * HC],
                                     start=st0, stop=sp1,
                                     skip_group_check=True)
                    nc.tensor.matmul(psm[:, :], lhsT=St(t),
                                     rhs=SMe[:, t * SMW:(t + 1) * SMW],
                                     start=st0, stop=sp1,
                                     skip_group_check=True)
                if debug and bi == 0:
                    dt = sp.tile([P, 68], F32, tag="dbgp", name="dbgp")
                    nc.vector.tensor_copy(dt[:, 0:SMW], psm[:, :])
                    nc.sync.dma_start(
                        dbg["dbg_psm1" if L == 1 else "dbg_psm2"][:, 0:SMW],
                        dt[:, 0:SMW])
                    dp = sp.tile([P, 2 * H], F32, tag="dbgd", name="dbgd")
                    nc.vector.tensor_copy(dp[:, 0:AUXW], pD[:, 0:AUXW])
                    if L == 1:
                        nc.sync.dma_start(dbg["dbg_pd1"][:, 0:AUXW],
                                          dp[:, 0:AUXW])
                # ---- drain ----
                if L == 1:
                    drain1(bi, bo, bs, pf, psm, sp, pp, pp2)
                else:
                    drain2(bi, bo, bs, pf, psm, sp, pp)

        # ---- L1 drain + fused L2 prep ----
        def drain1(bi, bo, bs, pf, psm, sp, pp, pp2):
            # rec2 = 1/sum(e2), rec1' = 1/(H*sum(e1))
            den = sp.tile([P, 2 * H], F32, tag="den", name="den")
            nc.vector.tensor_scalar(
                den[:, 0:H].unsqueeze(2),
                psm[:, 0:16 * H].rearrange("p (h j) -> p h j", j=16)[
                    :, :, 15:16],
                float(H), eps_col[:, 0:1], OP.mult, OP.max)
            nc.vector.tensor_tensor(den[:, H:2 * H], psm[:, 16 * H:16 * H + H],
                                    eps_col[:, 0:1].to_broadcast((P, H)),
                                    OP.max)
            rec = sp.tile([P, 2 * H], F32, tag="rec", name="rec")
            nc.vector.reciprocal(rec[:, :], den[:, :])
            # h1 = agg_feat * rec2 (node-major, bf16; per-head scale on Act)
            h1 = sp.tile([P, HC], BF16, tag="h1", name="h1")
            for h in range(H):
                nc.scalar.activation(h1[:, h * HID:(h + 1) * HID],
                                     pf[:, h * HID:(h + 1) * HID], AF.Copy,
                                     scale=rec[:, H + h:H + h + 1])
            # topo1_raw = sum_h agg_topo_h * rec1'   [d, 15]
            tp = sp.tile([P, TOPO * H], F32, tag="tp", name="tp")
            nc.vector.tensor_tensor(
                tp[:, :].rearrange("p (j h) -> p j h", h=H),
                psm[:, 0:16 * H].rearrange("p (h j) -> p h j", j=16)[
                    :, :, 0:TOPO].transpose([0, 2, 1]),
                rec[:, 0:H].unsqueeze(1).to_broadcast((P, TOPO, H)),
                OP.mult)
            t1 = sp.tile([P, TOPO], F32, tag="t1", name="t1")
            nc.vector.tensor_reduce(
                t1[:, :], tp[:, :].rearrange("p (j h) -> p j h", h=H),
                mybir.AxisListType.X, OP.add)
            # ta2 = topo1_raw @ att2T2 (per-node, via DVE reduce)
            tq = sp.tile([P, H * TOPO], F32, tag="tq", name="tq")
            nc.vector.tensor_tensor(
                tq[:, :].rearrange("p (h j) -> p h j", j=TOPO),
                t1[:, :].unsqueeze(1).to_broadcast((P, H, TOPO)),
                att2bc[:, :].rearrange("p (h j) -> p h j", j=TOPO),
                OP.mult)
            ta2 = sp.tile([P, H], F32, tag="ta2", name="ta2")
            nc.vector.tensor_reduce(
                ta2[:, :], tq[:, :].rearrange("p (h j) -> p h j", j=TOPO),
                mybir.AxisListType.X, OP.add)
            # dst rows for L2: ta2 + 2*(b2@att2)
            nc.vector.tensor_tensor(datt2[:bs, bi * H:(bi + 1) * H],
                                    ta2[:bs, :],
                                    ta2cbc[:bs, :], OP.add)
            if debug and bi == 0:
                dh = sp.tile([P, HC], F32, tag="dbgh", name="dbgh")
                nc.vector.tensor_copy(dh[:, :], h1[:, :])
                nc.sync.dma_start(dbg["dbg_h1"][:, :], dh[:, :])
                dtt = sp.tile([P, TOPO + H], F32, tag="dbgt", name="dbgt")
                nc.vector.tensor_copy(dtt[:, 0:TOPO], t1[:, :])
                nc.vector.tensor_copy(dtt[:, TOPO:TOPO + H], ta2[:, :])
                nc.sync.dma_start(dbg["dbg_tt"][:, :], dtt[:, :])
            # transpose h1 -> hfmT tiles
            for ci, (co, ck) in enumerate(ckt):
                ptr = pp.tile([P, P], BF16, tag="ptr", name="ptr",
                              space="PSUM")
                nc.tensor.transpose(ptr[:ck, :bs], h1[:bs, co:co + ck],
                                    ident_bf[:bs, :bs])
                nc.scalar.copy(hfmT[ci][:ck, bo:bo + bs], ptr[:ck, :bs])
            # ---- fused L2 prep for this block ----
            pm2 = pp2.tile([P, HC], F32, tag="pm2", name="pm2", space="PSUM")
            for ci, (co, ck) in enumerate(ckt):
                nc.tensor.matmul(pm2[:bs, :], lhsT=hfmT[ci][:ck, bo:bo + bs],
                                 rhs=wsb["wl2"][ci][:ck, :],
                                 start=ci == 0, stop=False,
                                 skip_group_check=True)
            nc.tensor.matmul(pm2[:bs, :], lhsT=ones_row[:, bo:bo + bs],
                             rhs=wsb["bl2"][:, :], start=False, stop=True,
                             skip_group_check=True)
            ext = sp.tile([P, ROW], F8, tag="ext2", name="ext2")
            nc.scalar.copy(ext[:bs, 0:HC], pm2[:bs, :])
            nc.scalar.copy(ext[:, :].bitcast(BF16)[
                :bs, cfg.C_TA2:cfg.C_TA2 + H], ta2[:bs, :])
            nc.sync.dma_start(ext_sl[1][bo:bo + bs, :], ext[:bs, :])

        # ---- L2 drain: normalize + pooled partial ----
        def drain2(bi, bo, bs, pf, psm, sp, pp):
            den = sp.tile([P, H], F32, tag="den2", name="den2")
            nc.vector.tensor_tensor(den[:, :], psm[:, 0:H],
                                    eps_col[:, 0:1].to_broadcast((P, H)),
                                    OP.max)
            rec = sp.tile([P, H], F32, tag="rec2", name="rec2")
            nc.vector.reciprocal(rec[:, :], den[:, :])
            h2 = sp.tile([P, HC], BF16, tag="h2", name="h2")
            for h in range(H):
                nc.scalar.activation(h2[:, h * HID:(h + 1) * HID],
                                     pf[:, h * HID:(h + 1) * HID], AF.Copy,
                                     scale=rec[:, h:h + 1])
            if debug and bi == 0:
                dh = sp.tile([P, HC], F32, tag="dbgh2", name="dbgh2")
                nc.vector.tensor_copy(dh[:, :], h2[:, :])
                nc.sync.dma_start(dbg["dbg_h2"][:, :], dh[:, :])
            nc.tensor.matmul(pr.pool_ps[:, :], lhsT=ones_col[:bs, 0:1],
                             rhs=h2[:bs, :], start=bi == 0,
                             stop=bi == len(blocks) - 1,
                             skip_group_check=True)

        # ================= phase B/C: L1 edges (+L2 prep) =================
        # PSUM banks: (pf+psm) 2x2 + pm2 2 + pD/ptr 1 each = 8 of 8
        with tc.tile_pool(name="gpB", bufs=3) as gpB, \
             tc.tile_pool(name="spB", bufs=3) as spB, \
             tc.tile_pool(name="ppB", bufs=1, space="PSUM") as ppB, \
             tc.tile_pool(name="ppB2", bufs=2, space="PSUM") as ppB2:
            emit_edge(1, gpB, spB, ppB, ppB2)
            nc.gpsimd.collective_compute(
                "AllGather", OP.bypass, replica_groups=groups,
                ins=[ext_sl[1][:, :]], outs=[ext_fl[1][:, :]])

        # ================= phase D: L2 edges =================
        with tc.tile_pool(name="gpD", bufs=3) as gpD, \
             tc.tile_pool(name="spD", bufs=3) as spD, \
             tc.tile_pool(name="ppD", bufs=1, space="PSUM") as ppD, \
             tc.tile_pool(name="ppD2", bufs=2, space="PSUM") as ppD2, \
             tc.tile_pool(name="plD", bufs=1, space="PSUM") as plD:
            pr.pool_ps = plD.tile([1, HC], F32, tag="pool", name="pool",
                                  space="PSUM", bufs=1)
            emit_edge(2, gpD, spD, ppD, ppD2)

            # ---- pool + heads ----
            pooled = spD.tile([1, HC], F32, tag="pooled", name="pooled")
            nc.vector.tensor_copy(pooled[:, :], pr.pool_ps[:, :])
            nc.sync.dma_start(pool_in[:, :], pooled[:, :])
            nc.gpsimd.collective_compute(
                "AllReduce", OP.add, replica_groups=groups,
                ins=[pool_in[:, :]], outs=[pool_out[:, :]])
            # load back column-major: pmean_cols[c, h] = pool_out[h*HID+c]
            pooled2 = spD.tile([P, H], F32, tag="pooled2", name="pooled2")
            with nc.allow_non_contiguous_dma("pool row -> col-major reload"):
                nc.sync.dma_start(
                    pooled2[:, :],
                    pool_out[:, :].rearrange("o (h c) -> (o c) h", c=HID))
            if debug:
                nc.sync.dma_start(dbg["dbg_pool"][:, :], pooled2[:, :])
            pmean = spD.tile([P, H], BF16, tag="pmean", name="pmean")
            nc.vector.tensor_scalar(pmean[:, :], pooled2[:, :], 1.0 / N,
                                    None, OP.mult)
            for nm, out_t in (("v", val_o), ("a", aro_o)):
                pm = ppD.tile([P, 1], F32, tag="mlp", name="mlp", space="PSUM")
                for ki in range(H):
                    nc.tensor.matmul(pm[:, :], lhsT=wsb[f"{nm}w1"][ki][:, :],
                                     rhs=pmean[:, ki:ki + 1], start=ki == 0,
                                     stop=False, skip_group_check=True)
                nc.tensor.matmul(pm[:, :], lhsT=wsb[f"{nm}b1"][:, :],
                                 rhs=ones_col[0:1, :], start=False, stop=True,
                                 skip_group_check=True)
                hv = spD.tile([P, 1], BF16, tag=f"{nm}hv", name=f"{nm}hv")
                nc.scalar.activation(hv[:, :], pm[:, :], AF.Relu)
                po = ppD.tile([1, 1], F32, tag="mlpo", name="mlpo",
                              space="PSUM")
                nc.tensor.matmul(po[:, :], lhsT=hv[:, :],
                                 rhs=wsb[f"{nm}w2"][:, :], start=True,
                                 stop=False, skip_group_check=True)
                nc.tensor.matmul(po[:, :], lhsT=wsb[f"{nm}b2"][:, :],
                                 rhs=ones_col[0:1, :], start=False, stop=True,
                                 skip_group_check=True)
                ov = spD.tile([1, 1], F32, tag=f"{nm}ov", name=f"{nm}ov")
                nc.vector.tensor_copy(ov[:, :], po[:, :])
                nc.sync.dma_start(out_t[:, :], ov[:, :])

    nc.compile()
    es.close()
    return pr


# --------------------------------------------------------------------------
# entry point
# --------------------------------------------------------------------------

_CACHE = {}


def make_in_maps(inputs, cfg, src_w, stt):
    x = np.asarray(inputs["x"], dtype=np.float32)
    shared = host_weights(inputs, cfg)
    in_maps = []
    for c in range(cfg.CORES):
        m = dict(shared)
        m["xT_slice"] = np.ascontiguousarray(
            x[c * cfg.NPC:(c + 1) * cfg.NPC].T.astype(BF))
        m["src_idx"] = np.ascontiguousarray(src_w[c])
        m["stt_tab"] = stt[c]
        in_maps.append(m)
    return in_maps


def run(inputs, cfg=CFG, trace=False):
    schedule, src_w, stt = host_prep(inputs["edge_index"], cfg)
    key = (cfg.N, cfg.E, tuple(schedule))
    if key not in _CACHE:
        _CACHE[key] = build_program(cfg, schedule)
    pr = _CACHE[key]
    in_maps = make_in_maps(inputs, cfg, src_w, stt)
    res = run_bass_kernel_spmd(pr.nc, in_maps, list(range(cfg.CORES)),
                               trace=trace)
    out = res.results[0]
    return (np.asarray(out["valence"], np.float32),
            np.asarray(out["arousal"], np.float32)), res


def kernel(**inputs):
    (val, aro), _ = run(inputs)
    return (val, aro)


# revision 59
# speedup vs baseline: 1.6148x; 1.0171x over previous
"""GCATopo (2-layer GTAT GNN) Trainium2 kernel, 8-way SPMD — v2.

Strategy (v2 redesign vs v1):
 - Node-major aggregation: per 128-edge tile ONE 512-wide matmul
   (lhsT=St one-hot, rhs=et2-weighted gathered features) accumulates
   [dst, 512] in a single PSUM bank; softmax denominators aggregate in a
   second small matmul. Normalization becomes per-partition scaling.
 - Per-edge dst logits come from a lookup matmul (lhsT=StT, rhs=local
   per-block dst-attn rows) instead of a 256B-per-edge DMA gather.
 - All per-edge elementwise work (logits, leaky-relu, exp, message
   weighting) is batched across a block's ~14 tiles with strided 3D/4D
   APs — a handful of DVE/Act instructions per block instead of ~15 per
   tile.
 - L2's topo output is discarded by the model, so L2 ships only
   [feat 512 | ta 4] and skips the SM stream entirely.
 - All matmul operands bf16 (4x PE rate vs f32); weights are host-folded
   (wl@attB etc.) and host-transposed; x arrives pre-transposed bf16.
 - Biases are folded forward into the next layer's constant rows, so
   drains are pure scaling.
 - L2 prep is fused into the L1 edge-phase block loop (PE prep matmuls
   overlap DVE/DMA edge work).
"""

from contextlib import ExitStack

import ml_dtypes
import numpy as np

import concourse.bacc as bacc
import concourse.tile as tile
from concourse import mybir
from concourse.masks import make_identity
from concourse.bass_utils import run_bass_kernel_spmd

F32 = mybir.dt.float32
BF16 = mybir.dt.bfloat16
F8 = mybir.dt.float8e4
I16 = mybir.dt.int16
AF = mybir.ActivationFunctionType
OP = mybir.AluOpType

P = 128
BF = ml_dtypes.bfloat16


class Cfg:
    def __init__(self, N=20000, E=240000, FIN=576, HID=128, TOPO=15, H=4,
                 CORES=8, NEG=0.2):
        self.N, self.E, self.FIN, self.HID, self.TOPO, self.H = N, E, FIN, HID, TOPO, H
        self.CORES, self.NEG = CORES, NEG
        self.HC = H * HID                      # 512
        self.ROW = 768                         # gathered row: fp8 feat + pad
        self.RB = self.ROW // 2                # bf16 view width (384)
        self.NPC = N // CORES                  # nodes per core
        self.NBLK = (self.NPC + P - 1) // P    # dst blocks per core
        # aux slots within the BF16 VIEW of the row (bf16 element offsets;
        # feat occupies bf16-view [0:256))
        self.C_TOPO = 256                      # 256..270: topo (L1)
        self.C_ONE = 256 + TOPO                # 271: constant 1.0 (L1)
        self.C_AL = 272                        # 272..275: al (L1)
        self.C_TA = 276                        # 276..279: ta (L1)
        self.C_TA2 = 256                       # 256..259: ta (L2)


CFG = Cfg()
GT_MAX = 8  # max tiles (=128 idxs each) per gather call
GP_POOL_PCT = 25   # percent of Gp tiles offloaded DVE -> gpsimd (L1)
GP_POOL_PCT2 = 15  # same for L2 (gpsimd busier there)


def cdiv(a, b):
    return (a + b - 1) // b


def ktiles(F):
    return [(o, min(P, F - o)) for o in range(0, F, P)]


# --------------------------------------------------------------------------
# host-side graph preprocessing (pure indexing on edge_index)
# --------------------------------------------------------------------------

def host_prep(edge_index, cfg):
    N, CORES, NPC, NBLK = cfg.N, cfg.CORES, cfg.NPC, cfg.NBLK
    src = np.asarray(edge_index[0], dtype=np.int64)
    dst = np.asarray(edge_index[1], dtype=np.int64)
    loops = np.arange(N, dtype=np.int64)
    src = np.concatenate([src, loops])
    dst = np.concatenate([dst, loops])
    order = np.argsort(dst, kind="stable")
    s, d = src[order], dst[order]

    core_of = d // NPC
    blk_of = (d % NPC) // P
    counts = np.zeros((CORES, NBLK), dtype=np.int64)
    for c in range(CORES):
        m = core_of == c
        bb = blk_of[m]
        for b in range(NBLK):
            counts[c, b] = int((bb == b).sum())
    schedule = [max(1, cdiv(int(counts[:, b].max()), P)) for b in range(NBLK)]
    offs = np.concatenate([[0], np.cumsum(schedule)]).astype(np.int64)
    ttot = int(offs[-1])

    srcidx = np.zeros((CORES, ttot * P), dtype=np.int16)
    dstloc = np.full((CORES, ttot * P), -1.0, dtype=np.float32)
    for c in range(CORES):
        m = core_of == c
        sc, dc, bc = s[m], d[m], blk_of[m]
        for b in range(NBLK):
            mb = bc == b
            n = int(mb.sum())
            base = int(offs[b]) * P
            srcidx[c, base:base + n] = sc[mb].astype(np.int16)
            dstloc[c, base:base + n] = (dc[mb] - (c * NPC + b * P)).astype(np.float32)

    # wrap for dma_gather: index i lives at [i % 16, i // 16]; the 16-row
    # block is replicated 8x along partitions (one stripe per gpsimd core)
    src_w = [np.tile(srcidx[c].reshape(-1, 16).T, (8, 1)).copy()
             for c in range(CORES)]
    # host-built one-hot selection tables, per tile [St | StT] (bf16 0/1):
    #   St[e, d] = (dstloc[e] == d), StT = St^T
    rng = np.arange(P, dtype=np.float32)
    stt = []
    for c in range(CORES):
        dl = dstloc[c].reshape(ttot, P)
        St = (dl[:, :, None] == rng[None, None, :])          # [t, e, d]
        tab = np.concatenate([St, St.transpose(0, 2, 1)], 2)  # [t, p, 256]
        stt.append(np.ascontiguousarray(
            tab.transpose(1, 0, 2).reshape(P, ttot * 2 * P).astype(BF)))
    return schedule, src_w, stt


def host_weights(inputs, cfg):
    """All small-weight folding in f32 numpy, shipped as bf16."""
    H, C, TOPO, HC = cfg.H, cfg.HID, cfg.TOPO, cfg.HC
    f = lambda k: np.asarray(inputs[k], np.float32)

    def attB(att):  # [1,H,C] -> block-diag [H*C, H]
        out = np.zeros((H * C, H), np.float32)
        a = np.asarray(att, np.float32).reshape(H, C)
        for h in range(H):
            out[h * C:(h + 1) * C, h] = a[h]
        return out

    w = {}
    # topo extractor
    w["tw1"] = f("te_w1")                      # [576,128]
    w["tb1"] = f("te_b1").reshape(1, -1)
    w["tw2"] = f("te_w2")                      # [128,15]
    w["tb2"] = f("te_b2").reshape(1, -1)
    # layer 1
    aB1 = attB(inputs["l1_att"])
    w["wl1"] = f("l1_wl")                      # [576,512]
    w["bl1"] = f("l1_bl").reshape(1, -1)
    w["A1"] = np.concatenate([f("l1_wl") @ aB1, f("l1_wr") @ aB1], 1)  # [576,8]
    w["bA1"] = np.concatenate([f("l1_bl") @ aB1, f("l1_br") @ aB1]).reshape(1, -1)
    w["att2T1"] = f("l1_att2").reshape(H, TOPO).T      # [15,4]
    # layer 2 (input h1 = agg1_norm, l1_bias folded here)
    b1 = f("l1_bias")
    w["wl2"] = f("l2_wl")                      # [512,512]
    w["bl2"] = (b1 @ f("l2_wl") + f("l2_bl")).reshape(1, -1)
    w["att2T2"] = f("l2_att2").reshape(H, TOPO).T      # [15,4]
    # topo1 input to L2 = topo1_raw + l1_bias2; edge logit gets the const
    # twice (src+dst) -> fold 2*(b2@att2) into the dst-side rows only
    w["ta2c"] = (2.0 * (f("l1_bias2") @ w["att2T2"])).reshape(1, -1)   # [1,4]
    # heads (l2_bias folded into first-layer bias)
    b2f = f("l2_bias")
    for nm in ("v", "a"):
        w[f"{nm}w1"] = f(f"{nm}_w1")           # [512,128]
        w[f"{nm}b1"] = (f(f"{nm}_b1") + b2f @ f(f"{nm}_w1")).reshape(1, -1)
        w[f"{nm}w2"] = f(f"{nm}_w2")           # [128,1]
        w[f"{nm}b2"] = f(f"{nm}_b2").reshape(1, 1)
    # att2T2 flattened (h,j) row for the drain's ta2 reduce + const
    w["att2f"] = w["att2T2"].T.reshape(1, -1)  # [1,60] (h-major)
    return {k: v.astype(BF) for k, v in w.items()}


# --------------------------------------------------------------------------
# program builder
# --------------------------------------------------------------------------

class Prog:
    pass


def build_program(cfg, schedule, debug=False):
    es = ExitStack()
    nc = bacc.Bacc("TRN2", target_bir_lowering=False, debug=False,
                   num_devices=cfg.CORES)
    pr = Prog()
    pr.nc = nc
    N, FIN, HID, TOPO, H, HC, ROW, NPC, NBLK = (
        cfg.N, cfg.FIN, cfg.HID, cfg.TOPO, cfg.H, cfg.HC, cfg.ROW, cfg.NPC,
        cfg.NBLK)
    TTOT = sum(schedule)
    W16 = TTOT * P // 16
    groups = [list(range(cfg.CORES))]
    blocks = ktiles(NPC)
    fkt = ktiles(FIN)
    ckt = ktiles(HC)
    offs = np.concatenate([[0], np.cumsum(schedule)]).astype(int)

    def din(name, shape, dtype=BF16):
        return nc.dram_tensor(name, list(shape), dtype, kind="ExternalInput")

    # ---- external inputs ----
    xT = din("xT_slice", (FIN, NPC))
    wnames = [("tw1", (FIN, HID)), ("tb1", (1, HID)), ("tw2", (HID, TOPO)),
              ("tb2", (1, TOPO)), ("wl1", (FIN, HC)), ("bl1", (1, HC)),
              ("A1", (FIN, 2 * H)), ("bA1", (1, 2 * H)), ("att2T1", (TOPO, H)),
              ("wl2", (HC, HC)), ("bl2", (1, HC)), ("att2T2", (TOPO, H)),
              ("ta2c", (1, H)), ("att2f", (1, H * TOPO)),
              ("vw1", (HC, HID)), ("vb1", (1, HID)), ("vw2", (HID, 1)),
              ("vb2", (1, 1)),
              ("aw1", (HC, HID)), ("ab1", (1, HID)), ("aw2", (HID, 1)),
              ("ab2", (1, 1))]
    W = {nm: din(nm, sh) for nm, sh in wnames}
    src_i = din("src_idx", (P, W16), I16)
    stt_i = din("stt_tab", (P, TTOT * 2 * P))

    # ---- outputs ----
    val_o = nc.dram_tensor("valence", [1, 1], F32, kind="ExternalOutput")
    aro_o = nc.dram_tensor("arousal", [1, 1], F32, kind="ExternalOutput")
    dbg = {}
    if debug:
        for nm, sh in [("dbg_h1", (P, HC)), ("dbg_tt", (P, TOPO + H)),
                       ("dbg_psm1", (P, 68)), ("dbg_h2", (P, HC)),
                       ("dbg_pool", (P, H)), ("dbg_aux", (P, 24)),
                       ("dbg_psm2", (P, H)), ("dbg_pd1", (P, 2 * H))]:
            dbg[nm] = nc.dram_tensor(nm, list(sh), F32, kind="ExternalOutput")

    # ---- internal DRAM ----
    ext_sl = [nc.dram_tensor(f"ext_slice{L}", [NPC, ROW], F8)
              for L in (1, 2)]
    ext_fl = [nc.dram_tensor(f"ext_full{L}", [N, ROW], F8,
                             addr_space="Shared") for L in (1, 2)]
    pool_in = nc.dram_tensor("pool_in", [1, HC], F32)
    pool_out = nc.dram_tensor("pool_out", [1, HC], F32, addr_space="Shared")

    with tile.TileContext(nc) as tc:
        # ================= static SBUF =================
        ident = nc.alloc_sbuf_tensor("ident", [P, P], F32).ap()
        make_identity(nc, ident)
        ones_row = nc.alloc_sbuf_tensor("ones_row", [1, NPC], BF16).ap()
        nc.gpsimd.memset(ones_row, 1.0)
        ones_col = nc.alloc_sbuf_tensor("ones_col", [P, 1], BF16).ap()
        nc.gpsimd.memset(ones_col, 1.0)
        eps_col = nc.alloc_sbuf_tensor("eps_col", [P, 1], F32).ap()
        nc.gpsimd.memset(eps_col, 1e-30)

        src_sb = nc.alloc_sbuf_tensor("src_sb", [P, W16], I16).ap()
        nc.sync.dma_start(src_sb, src_i[:, :])

        # resident activations / weights
        xT_sb = [nc.alloc_sbuf_tensor(f"xT{i}", [P, NPC], BF16).ap()
                 for i in range(len(fkt))]
        for i, (fo, fk) in enumerate(fkt):
            nc.sync.dma_start(xT_sb[i][:fk, :], xT[fo:fo + fk, :])
        hfmT = [nc.alloc_sbuf_tensor(f"hfmT{i}", [P, NPC], BF16).ap()
                for i in range(len(ckt))]
        topoT0 = nc.alloc_sbuf_tensor("topoT0", [TOPO, NPC], BF16).ap()
        datt1 = nc.alloc_sbuf_tensor("datt1", [P, NBLK * 2 * H], BF16).ap()
        datt2 = nc.alloc_sbuf_tensor("datt2", [P, NBLK * H], BF16).ap()
        nc.vector.memset(datt1, 0.0)   # rows past a partial block stay 0
        nc.vector.memset(datt2, 0.0)

        wsb = {}
        for nm, sh in wnames:
            if sh[0] <= P:
                wsb[nm] = nc.alloc_sbuf_tensor(f"w_{nm}", list(sh), BF16).ap()
                nc.sync.dma_start(wsb[nm], W[nm][:, :])
            else:  # k-tiled along the first (contraction) dim
                tiles = []
                for i, (fo, fk) in enumerate(ktiles(sh[0])):
                    t = nc.alloc_sbuf_tensor(f"w_{nm}{i}", [fk, sh[1]],
                                             BF16).ap()
                    nc.sync.dma_start(t, W[nm][fo:fo + fk, :])
                    tiles.append(t)
                wsb[nm] = tiles
        # att2f / ta2c broadcast to all partitions
        att2bc = nc.alloc_sbuf_tensor("att2bc", [P, H * TOPO], BF16).ap()
        nc.gpsimd.partition_broadcast(att2bc, wsb["att2f"][0:1, :])
        ta2cbc = nc.alloc_sbuf_tensor("ta2cbc", [P, H], BF16).ap()
        nc.gpsimd.partition_broadcast(ta2cbc, wsb["ta2c"][0:1, :])
        ident_bf = nc.alloc_sbuf_tensor("ident_bf", [P, P], BF16).ap()
        nc.vector.tensor_copy(ident_bf, ident)

        # ================= phase A: topo MLP + L1 prep =================
        with tc.tile_pool(name="ppA", bufs=1, space="PSUM") as ppA, \
             tc.tile_pool(name="ppA2", bufs=2, space="PSUM") as ppA2, \
             tc.tile_pool(name="cpA", bufs=3) as cpA, \
             tc.tile_pool(name="spA", bufs=2) as spA:
            # --- topo extractor MLP (feat-major: out rows = hid/topo) ---
            NG = 512
            for go in range(0, NPC, NG):
                gs = min(NG, NPC - go)
                ph = ppA.tile([P, NG], F32, tag="ph", name="ph", space="PSUM")
                for i, (fo, fk) in enumerate(fkt):
                    nc.tensor.matmul(ph[:, :gs], lhsT=wsb["tw1"][i][:fk, :],
                                     rhs=xT_sb[i][:fk, go:go + gs],
                                     start=i == 0, stop=False,
                                     skip_group_check=True)
                nc.tensor.matmul(ph[:, :gs], lhsT=wsb["tb1"][:, :],
                                 rhs=ones_row[:, go:go + gs], start=False,
                                 stop=True, skip_group_check=True)
                t_hid = spA.tile([P, NG], BF16, tag="t_hid", name="t_hid")
                nc.scalar.activation(t_hid[:, :gs], ph[:, :gs], AF.Relu)
                pt = ppA.tile([TOPO, NG], F32, tag="pt", name="pt", space="PSUM")
                nc.tensor.matmul(pt[:, :gs], lhsT=wsb["tw2"][:, :],
                                 rhs=t_hid[:, :gs], start=True, stop=False,
                                 skip_group_check=True)
                nc.tensor.matmul(pt[:, :gs], lhsT=wsb["tb2"][:, :],
                                 rhs=ones_row[:, go:go + gs], start=False,
                                 stop=True, skip_group_check=True)
                nc.vector.tensor_copy(topoT0[:, go:go + gs], pt[:, :gs])

            # --- L1 prep per block ---
            for bi, (bo, bs) in enumerate(blocks):
                pm = ppA2.tile([P, HC], F32, tag="pm", name="pm", space="PSUM")
                pa = ppA.tile([P, 2 * H], F32, tag="pa", name="pa", space="PSUM")
                for i, (fo, fk) in enumerate(fkt):
                    nc.tensor.matmul(pm[:bs, :], lhsT=xT_sb[i][:fk, bo:bo + bs],
                                     rhs=wsb["wl1"][i][:fk, :],
                                     start=i == 0, stop=False,
                                     skip_group_check=True)
                    nc.tensor.matmul(pa[:bs, :], lhsT=xT_sb[i][:fk, bo:bo + bs],
                                     rhs=wsb["A1"][i][:fk, :],
                                     start=i == 0, stop=False,
                                     skip_group_check=True)
                nc.tensor.matmul(pm[:bs, :], lhsT=ones_row[:, bo:bo + bs],
                                 rhs=wsb["bl1"][:, :], start=False, stop=True,
                                 skip_group_check=True)
                nc.tensor.matmul(pa[:bs, :], lhsT=ones_row[:, bo:bo + bs],
                                 rhs=wsb["bA1"][:, :], start=False, stop=True,
                                 skip_group_check=True)
                pta = ppA.tile([P, H], F32, tag="pta", name="pta", space="PSUM")
                nc.tensor.matmul(pta[:bs, :], lhsT=topoT0[:, bo:bo + bs],
                                 rhs=wsb["att2T1"][:, :], start=True,
                                 stop=True, skip_group_check=True)
                ptt = ppA.tile([P, TOPO], BF16, tag="ptt", name="ptt",
                               space="PSUM")
                nc.tensor.transpose(ptt[:bs, :TOPO],
                                    topoT0[:, bo:bo + bs],
                                    ident_bf[:TOPO, :TOPO])
                ext = cpA.tile([P, ROW], F8, tag="ext", name="ext")
                extb = ext[:, :].bitcast(BF16)
                nc.scalar.copy(ext[:bs, 0:HC], pm[:bs, :])
                nc.scalar.copy(extb[:bs, cfg.C_TOPO:cfg.C_TOPO + TOPO],
                               ptt[:bs, :TOPO])
                nc.vector.memset(extb[:bs, cfg.C_ONE:cfg.C_ONE + 1], 1.0)
                nc.scalar.copy(extb[:bs, cfg.C_AL:cfg.C_AL + H], pa[:bs, 0:H])
                nc.scalar.copy(extb[:bs, cfg.C_TA:cfg.C_TA + H], pta[:bs, :])
                nc.sync.dma_start(ext_sl[0][bo:bo + bs, :], ext[:bs, :])
                if debug and bi == 0:
                    da = cpA.tile([P, 24], F32, tag="dbga", name="dbga")
                    nc.vector.tensor_copy(da[:, :],
                                          extb[:, cfg.C_TOPO:cfg.C_TOPO + 24])
                    nc.sync.dma_start(dbg["dbg_aux"][:, :], da[:, :])
                # dst-side rows: [ar | ta]
                nc.vector.tensor_copy(datt1[:bs, bi * 2 * H:bi * 2 * H + H],
                                      pa[:bs, H:2 * H])
                nc.vector.tensor_copy(
                    datt1[:bs, bi * 2 * H + H:(bi + 1) * 2 * H], pta[:bs, :])
            nc.gpsimd.collective_compute(
                "AllGather", OP.bypass, replica_groups=groups,
                ins=[ext_sl[0][:, :]], outs=[ext_fl[0][:, :]])

        # ================= edge phase (shared emitter) =================
        TMAX = max(schedule)

        def emit_edge(L, gp, sp, pp, pp2):
            AUXW = 2 * H if L == 1 else H      # lg width per tile
            AUXO = cfg.C_AL if L == 1 else cfg.C_TA2
            for bi, (bo, bs) in enumerate(blocks):
                Tb = schedule[bi]
                base = int(offs[bi])
                TW = Tb * P
                # ---- gathers ----
                G = gp.tile([P, TMAX * ROW], F8, tag="G", name="G")
                for go in range(0, Tb, GT_MAX):
                    gn = min(GT_MAX, Tb - go)
                    c0 = (base + go) * 8
                    nc.gpsimd.dma_gather(
                        G[:, go * ROW:(go + gn) * ROW].rearrange(
                            "p (t e) -> p t e", e=ROW),
                        ext_fl[L - 1][:, :], src_sb[:, c0:c0 + 8 * gn],
                        num_idxs=P * gn, num_idxs_reg=P * gn, elem_size=ROW,
                        queue_num=0)
                # ---- St / StT (host-built one-hot tables) ----
                stt = sp.tile([P, TMAX * 2 * P], BF16, tag="stt", name="stt")
                nc.sync.dma_start(stt[:, 0:Tb * 2 * P],
                                  stt_i[:, base * 2 * P:(base + Tb) * 2 * P])

                def St(t):
                    return stt[:, t * 2 * P:t * 2 * P + P]

                def StT(t):
                    return stt[:, t * 2 * P + P:(t + 1) * 2 * P]
                # ---- dst-logit lookup ----
                pD = pp.tile([P, TMAX * AUXW], F32, tag="pD", name="pD",
                             space="PSUM")
                dsl = (datt1[:, bi * 2 * H:(bi + 1) * 2 * H] if L == 1
                       else datt2[:, bi * H:(bi + 1) * H])
                for t in range(Tb):
                    nc.tensor.matmul(pD[:, t * AUXW:(t + 1) * AUXW],
                                     lhsT=StT(t), rhs=dsl, start=True,
                                     stop=True, skip_group_check=True)
                # ---- batched logits ----
                Gb = G[:, 0:Tb * ROW].bitcast(BF16).rearrange(
                    "p (t e) -> p t e", e=cfg.RB)
                lg = sp.tile([P, TMAX * AUXW], F32, tag="lg", name="lg")
                nc.vector.tensor_tensor(
                    lg[:, 0:Tb * AUXW].rearrange("p (t c) -> p t c", c=AUXW),
                    Gb[:, :, AUXO:AUXO + AUXW],
                    pD[:, 0:Tb * AUXW].rearrange("p (t c) -> p t c", c=AUXW),
                    OP.add)
                lr = sp.tile([P, TMAX * AUXW], F32, tag="lr", name="lr")
                nc.vector.scalar_tensor_tensor(
                    lr[:, 0:Tb * AUXW], lg[:, 0:Tb * AUXW], cfg.NEG,
                    lg[:, 0:Tb * AUXW], OP.mult, OP.max)
                et = sp.tile([P, TMAX * AUXW], BF16, tag="et", name="et")
                nc.scalar.activation(et[:, 0:Tb * AUXW], lr[:, 0:Tb * AUXW],
                                     AF.Exp)
                etv = et[:, 0:Tb * AUXW].rearrange("p (t c) -> p t c", c=AUXW)
                # ---- weighted messages (split DVE / gpsimd) ----
                Gp = gp.tile([P, TMAX * HC], BF16, tag="Gp", name="Gp")
                e2off = H if L == 1 else 0
                Gf = G[:, 0:Tb * ROW].rearrange("p (t e) -> p t e", e=ROW)
                pct = GP_POOL_PCT if L == 1 else GP_POOL_PCT2
                ks = (Tb * pct + 99) // 100      # first ks tiles on Pool

                def gp_op(eng, t0, t1):
                    if t1 <= t0:
                        return
                    eng.tensor_tensor(
                        Gp[:, t0 * HC:t1 * HC].rearrange(
                            "p (t h c) -> p t h c", h=H, c=HID),
                        Gf[:, t0:t1, 0:HC].rearrange(
                            "p t (h c) -> p t h c", c=HID),
                        etv[:, t0:t1, e2off:e2off + H].unsqueeze(
                            3).to_broadcast((P, t1 - t0, H, HID)),
                        OP.mult)
                gp_op(nc.gpsimd, 0, ks)
                gp_op(nc.vector, ks, Tb)
                if L == 1:
                    SMW = 16 * H + H
                    SMe = sp.tile([P, TMAX * SMW], BF16, tag="SMe", name="SMe")
                    SMv = SMe[:, 0:Tb * SMW].rearrange("p (t c) -> p t c",
                                                       c=SMW)
                    nc.vector.tensor_tensor(
                        SMv[:, :, 0:16 * H].rearrange(
                            "p t (h j) -> p t h j", j=16),
                        Gb[:, :, cfg.C_TOPO:cfg.C_TOPO + 16].unsqueeze(
                            2).to_broadcast((P, Tb, H, 16)),
                        etv[:, :, 0:H].unsqueeze(3).to_broadcast(
                            (P, Tb, H, 16)),
                        OP.mult)
                    nc.scalar.copy(SMv[:, :, 16 * H:SMW],
                                   etv[:, :, H:2 * H])
                else:
                    SMW = H
                    SMe = et
                # ---- aggregation matmuls ----
                pf = pp2.tile([P, HC], F32, tag="pf", name="pf", space="PSUM")
                psm = pp2.tile([P, SMW], F32, tag="psm", name="psm",
                               space="PSUM")
                for t in range(Tb):
                    st0, sp1 = t == 0, t == Tb - 1
                    nc.tensor.matmul(pf[:, :], lhsT=St(t),
                                     rhs=Gp[:, t * HC:(t + 1) * HC],
                                     start=st0, stop=sp1,
                                     skip_group_check=True)
                    nc.tensor.matmul(psm[:, :], lhsT=St(t),
                                     rhs=SMe[:, t * SMW:(t + 1) * SMW],
                                     start=st0, stop=sp1,
                                     skip_group_check=True)
                if debug and bi == 0:
                    dt = sp.tile([P, 68], F32, tag="dbgp", name="dbgp")
                    nc.vector.tensor_copy(dt[:, 0:SMW], psm[:, :])
                    nc.sync.dma_start(
                        dbg["dbg_psm1" if L == 1 else "dbg_psm2"][:, 0:SMW],
                        dt[:, 0:SMW])
                    dp = sp.tile([P, 2 * H], F32, tag="dbgd", name="dbgd")
                    nc.vector.tensor_copy(dp[:, 0:AUXW], pD[:, 0:AUXW])
                    if L == 1:
                        nc.sync.dma_start(dbg["dbg_pd1"][:, 0:AUXW],
                                          dp[:, 0:AUXW])
                # ---- drain ----
                if L == 1:
                    drain1(bi, bo, bs, pf, psm, sp, pp, pp2)
                else:
                    drain2(bi, bo, bs, pf, psm, sp, pp)

        # ---- L1 drain + fused L2 prep ----
        def drain1(bi, bo, bs, pf, psm, sp, pp, pp2):
            # rec2 = 1/sum(e2), rec1' = 1/(H*sum(e1))
            den = sp.tile([P, 2 * H], F32, tag="den", name="den")
            nc.vector.tensor_scalar(
                den[:, 0:H].unsqueeze(2),
                psm[:, 0:16 * H].rearrange("p (h j) -> p h j", j=16)[
                    :, :, 15:16],
                float(H), eps_col[:, 0:1], OP.mult, OP.max)
            nc.vector.tensor_tensor(den[:, H:2 * H], psm[:, 16 * H:16 * H + H],
                                    eps_col[:, 0:1].to_broadcast((P, H)),
                                    OP.max)
            rec = sp.tile([P, 2 * H], F32, tag="rec", name="rec")
            nc.vector.reciprocal(rec[:, :], den[:, :])
            # h1 = agg_feat * rec2 (node-major, bf16; per-head scale on Act)
            h1 = sp.tile([P, HC], BF16, tag="h1", name="h1")
            for h in range(H):
                nc.scalar.activation(h1[:, h * HID:(h + 1) * HID],
                                     pf[:, h * HID:(h + 1) * HID], AF.Copy,
                                     scale=rec[:, H + h:H + h + 1])
            # topo1_raw = sum_h agg_topo_h * rec1'   [d, 15]
            tp = sp.tile([P, TOPO * H], F32, tag="tp", name="tp")
            nc.vector.tensor_tensor(
                tp[:, :].rearrange("p (j h) -> p j h", h=H),
                psm[:, 0:16 * H].rearrange("p (h j) -> p h j", j=16)[
                    :, :, 0:TOPO].transpose([0, 2, 1]),
                rec[:, 0:H].unsqueeze(1).to_broadcast((P, TOPO, H)),
                OP.mult)
            t1 = sp.tile([P, TOPO], F32, tag="t1", name="t1")
            nc.vector.tensor_reduce(
                t1[:, :], tp[:, :].rearrange("p (j h) -> p j h", h=H),
                mybir.AxisListType.X, OP.add)
            # ta2 = topo1_raw @ att2T2 (per-node, via DVE reduce)
            tq = sp.tile([P, H * TOPO], F32, tag="tq", name="tq")
            nc.vector.tensor_tensor(
                tq[:, :].rearrange("p (h j) -> p h j", j=TOPO),
                t1[:, :].unsqueeze(1).to_broadcast((P, H, TOPO)),
                att2bc[:, :].rearrange("p (h j) -> p h j", j=TOPO),
                OP.mult)
            ta2 = sp.tile([P, H], F32, tag="ta2", name="ta2")
            nc.vector.tensor_reduce(
                ta2[:, :], tq[:, :].rearrange("p (h j) -> p h j", j=TOPO),
                mybir.AxisListType.X, OP.add)
            # dst rows for L2: ta2 + 2*(b2@att2)
            nc.vector.tensor_tensor(datt2[:bs, bi * H:(bi + 1) * H],
                                    ta2[:bs, :],
                                    ta2cbc[:bs, :], OP.add)
            if debug and bi == 0:
                dh = sp.tile([P, HC], F32, tag="dbgh", name="dbgh")
                nc.vector.tensor_copy(dh[:, :], h1[:, :])
                nc.sync.dma_start(dbg["dbg_h1"][:, :], dh[:, :])
                dtt = sp.tile([P, TOPO + H], F32, tag="dbgt", name="dbgt")
                nc.vector.tensor_copy(dtt[:, 0:TOPO], t1[:, :])
                nc.vector.tensor_copy(dtt[:, TOPO:TOPO + H], ta2[:, :])
                nc.sync.dma_start(dbg["dbg_tt"][:, :], dtt[:, :])
            # transpose h1 -> hfmT tiles
            for ci, (co, ck) in enumerate(ckt):
                ptr = pp.tile([P, P], BF16, tag="ptr", name="ptr",
                              space="PSUM")
                nc.tensor.transpose(ptr[:ck, :bs], h1[:bs, co:co + ck],
                                    ident_bf[:bs, :bs])
                nc.scalar.copy(hfmT[ci][:ck, bo:bo + bs], ptr[:ck, :bs])
            # ---- fused L2 prep for this block ----
            pm2 = pp2.tile([P, HC], F32, tag="pm2", name="pm2", space="PSUM")
            for ci, (co, ck) in enumerate(ckt):
                nc.tensor.matmul(pm2[:bs, :], lhsT=hfmT[ci][:ck, bo:bo + bs],
                                 rhs=wsb["wl2"][ci][:ck, :],
                                 start=ci == 0, stop=False,
                                 skip_group_check=True)
            nc.tensor.matmul(pm2[:bs, :], lhsT=ones_row[:, bo:bo + bs],
                             rhs=wsb["bl2"][:, :], start=False, stop=True,
                             skip_group_check=True)
            ext = sp.tile([P, ROW], F8, tag="ext2", name="ext2")
            nc.scalar.copy(ext[:bs, 0:HC], pm2[:bs, :])
            nc.scalar.copy(ext[:, :].bitcast(BF16)[
                :bs, cfg.C_TA2:cfg.C_TA2 + H], ta2[:bs, :])
            nc.sync.dma_start(ext_sl[1][bo:bo + bs, :], ext[:bs, :])

        # ---- L2 drain: normalize + pooled partial ----
        def drain2(bi, bo, bs, pf, psm, sp, pp):
            den = sp.tile([P, H], F32, tag="den2", name="den2")
            nc.vector.tensor_tensor(den[:, :], psm[:, 0:H],
                                    eps_col[:, 0:1].to_broadcast((P, H)),
                                    OP.max)
            rec = sp.tile([P, H], F32, tag="rec2", name="rec2")
            nc.vector.reciprocal(rec[:, :], den[:, :])
            h2 = sp.tile([P, HC], BF16, tag="h2", name="h2")
            for h in range(H):
                nc.scalar.activation(h2[:, h * HID:(h + 1) * HID],
                                     pf[:, h * HID:(h + 1) * HID], AF.Copy,
                                     scale=rec[:, h:h + 1])
            if debug and bi == 0:
                dh = sp.tile([P, HC], F32, tag="dbgh2", name="dbgh2")
                nc.vector.tensor_copy(dh[:, :], h2[:, :])
                nc.sync.dma_start(dbg["dbg_h2"][:, :], dh[:, :])
            nc.tensor.matmul(pr.pool_ps[:, :], lhsT=ones_col[:bs, 0:1],
                             rhs=h2[:bs, :], start=bi == 0,
                             stop=bi == len(blocks) - 1,
                             skip_group_check=True)

        # ================= phase B/C: L1 edges (+L2 prep) =================
        # PSUM banks: (pf+psm) 2x2 + pm2 2 + pD/ptr 1 each = 8 of 8
        with tc.tile_pool(name="gpB", bufs=3) as gpB, \
             tc.tile_pool(name="spB", bufs=3) as spB, \
             tc.tile_pool(name="ppB", bufs=1, space="PSUM") as ppB, \
             tc.tile_pool(name="ppB2", bufs=2, space="PSUM") as ppB2:
            emit_edge(1, gpB, spB, ppB, ppB2)
            nc.gpsimd.collective_compute(
                "AllGather", OP.bypass, replica_groups=groups,
                ins=[ext_sl[1][:, :]], outs=[ext_fl[1][:, :]])

        # ================= phase D: L2 edges =================
        with tc.tile_pool(name="gpD", bufs=3) as gpD, \
             tc.tile_pool(name="spD", bufs=3) as spD, \
             tc.tile_pool(name="ppD", bufs=1, space="PSUM") as ppD, \
             tc.tile_pool(name="ppD2", bufs=2, space="PSUM") as ppD2, \
             tc.tile_pool(name="plD", bufs=1, space="PSUM") as plD:
            pr.pool_ps = plD.tile([1, HC], F32, tag="pool", name="pool",
                                  space="PSUM", bufs=1)
            emit_edge(2, gpD, spD, ppD, ppD2)

            # ---- pool + heads ----
            pooled = spD.tile([1, HC], F32, tag="pooled", name="pooled")
            nc.vector.tensor_copy(pooled[:, :], pr.pool_ps[:, :])
            nc.sync.dma_start(pool_in[:, :], pooled[:, :])
            nc.gpsimd.collective_compute(
                "AllReduce", OP.add, replica_groups=groups,
                ins=[pool_in[:, :]], outs=[pool_out[:, :]])
            # load back column-major: pmean_cols[c, h] = pool_out[h*HID+c]
            pooled2 = spD.tile([P, H], F32, tag="pooled2", name="pooled2")
            with nc.allow_non_contiguous_dma("pool row -> col-major reload"):
                nc.sync.dma_start(
                    pooled2[:, :],
                    pool_out[:, :].rearrange("o (h c) -> (o c) h", c=HID))
            if debug:
                nc.sync.dma_start(dbg["dbg_pool"][:, :], pooled2[:, :])
            pmean = spD.tile([P, H], BF16, tag="pmean", name="pmean")
            nc.vector.tensor_scalar(pmean[:, :], pooled2[:, :], 1.0 / N,
                                    None, OP.mult)
            for nm, out_t in (("v", val_o), ("a", aro_o)):
                pm = ppD.tile([P, 1], F32, tag="mlp", name="mlp", space="PSUM")
                for ki in range(H):
                    nc.tensor.matmul(pm[:, :], lhsT=wsb[f"{nm}w1"][ki][:, :],
                                     rhs=pmean[:, ki:ki + 1], start=ki == 0,
                                     stop=False, skip_group_check=True)
                nc.tensor.matmul(pm[:, :], lhsT=wsb[f"{nm}b1"][:, :],
                                 rhs=ones_col[0:1, :], start=False, stop=True,
                                 skip_group_check=True)
                hv = spD.tile([P, 1], BF16, tag=f"{nm}hv", name=f"{nm}hv")
                nc.scalar.activation(hv[:, :], pm[:, :], AF.Relu)
                po = ppD.tile([1, 1], F32, tag="mlpo", name="mlpo",
                              space="PSUM")
                nc.tensor.matmul(po[:, :], lhsT=hv[:, :],
                                 rhs=wsb[f"{nm}w2"][:, :], start=True,
                                 stop=False, skip_group_check=True)
                nc.tensor.matmul(po[:, :], lhsT=wsb[f"{nm}b2"][:, :],
                                 rhs=ones_col[0:1, :], start=False, stop=True,
                                 skip_group_check=True)
                ov = spD.tile([1, 1], F32, tag=f"{nm}ov", name=f"{nm}ov")
                nc.vector.tensor_copy(ov[:, :], po[:, :])
                nc.sync.dma_start(out_t[:, :], ov[:, :])

    nc.compile()
    es.close()
    return pr


# --------------------------------------------------------------------------
# entry point
# --------------------------------------------------------------------------

_CACHE = {}


def make_in_maps(inputs, cfg, src_w, stt):
    x = np.asarray(inputs["x"], dtype=np.float32)
    shared = host_weights(inputs, cfg)
    in_maps = []
    for c in range(cfg.CORES):
        m = dict(shared)
        m["xT_slice"] = np.ascontiguousarray(
            x[c * cfg.NPC:(c + 1) * cfg.NPC].T.astype(BF))
        m["src_idx"] = np.ascontiguousarray(src_w[c])
        m["stt_tab"] = stt[c]
        in_maps.append(m)
    return in_maps


def run(inputs, cfg=CFG, trace=False):
    schedule, src_w, stt = host_prep(inputs["edge_index"], cfg)
    key = (cfg.N, cfg.E, tuple(schedule))
    if key not in _CACHE:
        _CACHE[key] = build_program(cfg, schedule)
    pr = _CACHE[key]
    in_maps = make_in_maps(inputs, cfg, src_w, stt)
    res = run_bass_kernel_spmd(pr.nc, in_maps, list(range(cfg.CORES)),
                               trace=trace)
    out = res.results[0]
    return (np.asarray(out["valence"], np.float32),
            np.asarray(out["arousal"], np.float32)), res


def kernel(**inputs):
    (val, aro), _ = run(inputs)
    return (val, aro)
